# revision 4
# baseline (speedup 1.0000x reference)
"""CoulombLayer Trainium2 kernel v2 (8 NeuronCores, SPMD via bass).

Strategy vs v1 (baseline):
  * Edges are split on host into NEAR (d < 5, inside the smooth-cutoff
    transition of f(2d) with cutoff=10) and FAR (d >= 5, where the PhysNet
    cutoff f is exactly 0 so chi(d) = 1/d exactly).  69% of edges are far
    and need only a reciprocal and a multiply - no sqrt / smoothstep math.
  * Each stream gets its own vertex-cut CSR layout (degree-sorted atoms,
    dealt round-robin to cores, chunked with shared slot width K_c), its own
    accumulator grid and its own output; the host adds the two grids while
    unsharding (index-driven gather it already does anyway).
  * Reciprocals run as: ACT seed y = exp(-ln x) (natural_log_exp table set)
    + ONE fused Newton step on DVE via the custom-DVE op
    RECIPROCAL_APPROX_NR: ir = (2 - x*y)*y  (1 instruction instead of 2).
  * The near smoothstep g = f(z)*dm, f = z^3*((sqrt6 z - c)^2 + 0.625), is
    ONE authored custom-DVE instruction (F_SMOOTH_MUL, 8 ALU stages), with
    z = relu(1 - d/5) from ACT and dm = d - s from GPSIMD.
  * Engine balance per core (est): DVE ~52us, ACT ~50us, GPSIMD ~49us,
    DMA ~48us (fp32 memory floor).  Everything fp32: the 2e-2 max-rel-err
    gate is dominated by atoms with catastrophic term cancellation, which
    demands ~1e-7 per-term accuracy - no fp16/bf16 and no approximations
    beyond few-ulp.
  * ACT table sets: phase 1 (near s/z/p math) uses sqrt_and_others; phase 2
    (all seeds) uses natural_log_exp_and_others -> exactly 2 table loads per
    execution.
"""

import json as _json
import numpy as np

N_CORES = 8
N_ATOMS = 500_000
N_MOL = 5_000
N_EDGES = 16_000_000
CUTOFF = 10.0
P = 125                 # SBUF partitions used (125 * 500 = 62500 atoms/core)
APP = 500               # atoms per partition
CAT = 25                # atoms per compute tile (per partition)
GCAP = 2050             # max elements per grouped elementwise op

SQ6 = 6.0 ** 0.5
C15 = 15.0 / (2.0 * SQ6)

_RUNNER_CACHE = {}


# ---------------------------------------------------------------------------
# authored custom DVE ops (registered into concourse.dve_ops at first use):
#   F_SMOOTH_MUL_ANT: g = f(z)*dm, f = ((sqrt6*z - c)^2 + 0.625)*z^3 (the
#     PhysNet quintic smoothstep in z = relu(1 - d/5) form). 8 ALU stages,
#     one DVE pass instead of TT+STT+TT plus two ACT squares.
#   RECIP_NR2_ANT: two fused Newton steps y <- y*(2 - x*y) (6 stages). From
#     the ~1.2e-3 ACT exp(-ln x) seed this converges to the fp32 rounding
#     floor (~1.4e-7) in ONE DVE pass - the single-NR version left ~1e-6
#     per-term error, which the max-rel-err gate amplifies via cancellation.
# ---------------------------------------------------------------------------
_CUSTOM_OPS = None


def _get_custom_ops():
    global _CUSTOM_OPS
    if _CUSTOM_OPS is not None:
        return _CUSTOM_OPS
    import concourse.dve_ops as dve_ops
    from concourse.dve_spec import Spec, Src0, Src1, C0, C1, C2, lower, sq
    from concourse.dve_uop import DveOpSpec

    def _register(name, spec):
        if name in dve_ops._SUB_OPCODE_FOR_NAME:
            return next(op for op in dve_ops.OPS if op.name == name)
        row = max(dve_ops._SUB_OPCODE_FOR_NAME.values()) + 1
        assert row < 0x20
        shas = {}
        for ver in ("v3", "v4"):
            uops = lower(spec, ver=ver)
            shas[ver] = DveOpSpec(name=name, opcode=row, uops=uops,
                                  rd1_en=True).sha(ver)
        op = dve_ops.DveOp(name=name, spec=spec, subdim=False, uops_sha=shas)
        dve_ops._SUB_OPCODE_FOR_NAME[name] = row
        dve_ops.OPS.append(op)
        dve_ops.CUSTOM_DVE_SPECS[name] = spec
        return op

    def _fmul_ref(in0, in1, s0, s1, imm2):
        z = in0.astype(np.float32)
        f = ((z * s0 - s1) ** 2 + imm2) * z * z * z
        return (f * in1).astype(np.float32)

    fmul = _register("F_SMOOTH_MUL_ANT", Spec(
        body=(sq(Src0 * C0 - C1) + C2) * sq(Src0) * Src0 * Src1,
        reference=_fmul_ref,
    ))

    def _nr2_ref(in0, in1, s0, s1, imm2):
        t = ((s0 - in0 * in1) * in1).astype(np.float32)
        return ((s0 - in0 * t) * t).astype(np.float32)

    _y1 = (C0 - Src0 * Src1) * Src1
    nr2 = _register("RECIP_NR2_ANT", Spec(
        body=(C0 - Src0 * _y1) * _y1,
        reference=_nr2_ref,
    ))
    _CUSTOM_OPS = (fmul, nr2)
    return _CUSTOM_OPS


# ---------------------------------------------------------------------------
# walrus compat: this build rejects >1 sync-wait per instruction.  Split
# overflow waits onto NoOps inserted immediately before, same engine/block.
# ---------------------------------------------------------------------------
def _fix_bir_json(bir_json):
    m = _json.loads(bir_json)
    for fn in m.get("functions", []):
        for blk in fn.get("blocks", []):
            out = []
            for inst in blk.get("instructions", []):
                si = inst.get("sync_info")
                waits = (si or {}).get("on_wait", [])
                if len(waits) > 1:
                    for k, w in enumerate(waits[:-1]):
                        out.append({
                            "debug": inst.get("debug", 0),
                            "engine": inst["engine"],
                            "ins": [],
                            "name": f"{inst['name']}-sw{k}",
                            "opcode": "NoOp",
                            "outs": [],
                            "sync_info": {"on_update": [], "on_wait": [w]},
                        })
                    si["on_wait"] = [waits[-1]]
                out.append(inst)
            blk["instructions"] = out
    return _json.dumps(m).encode()


_PATCHED = False


def _install_compat():
    global _PATCHED
    if _PATCHED:
        return
    _PATCHED = True
    import concourse.bass_utils as bu
    import concourse.bass2jax as b2j
    orig = bu.compile_bir_kernel

    def patched(bir_json, tmpdir, neff_name="file.neff"):
        return orig(_fix_bir_json(bir_json), tmpdir, neff_name)

    bu.compile_bir_kernel = patched
    b2j.compile_bir_kernel = patched


def _groups(Ks):
    """Greedy-pack consecutive chunks into groups of total width <= GCAP."""
    out = []
    cur = [0]
    w = CAT * Ks[0]
    for c in range(1, len(Ks)):
        f = CAT * Ks[c]
        if w + f <= GCAP:
            cur.append(c)
            w += f
        else:
            out.append(cur)
            cur = [c]
            w = f
    out.append(cur)
    return out


# ---------------------------------------------------------------------------
# device program
# ---------------------------------------------------------------------------
def _build_nc(KsN, KsF, reps=1, body_mult=1,
              p_on_act=False, dm_on_gps=True, yn_on_gps=True, yf_on_gps=True,
              seed_dve=False, dma_only=False, tiny_dma=False,
              io_bufs=2, tmp_bufs=2):
    """Two-stream device program for one core.

    KsN/KsF: per-chunk slot widths for the near / far CSR layouts."""
    import concourse.bass as bass
    import concourse.mybir as mybir
    import concourse.tile as tile

    fmul, nr2 = _get_custom_ops()

    WN = sum(CAT * k for k in KsN)
    WF = sum(CAT * k for k in KsF)
    offsN = np.cumsum([0] + [CAT * k for k in KsN])
    offsF = np.cumsum([0] + [CAT * k for k in KsF])
    nc = bass.Bass()

    # const-AP pool entries for non-Copy activation biases (only 0.0/1.0 are
    # pre-registered)
    need_consts = [-0.25, 0.5] if p_on_act else []
    for v in need_consts:
        if (mybir.dt.float32, v) not in nc.const_aps.aps:
            ct = nc.alloc_sbuf_tensor(f"const-float32-{v}", [128, 1],
                                      mybir.dt.float32)
            nc.gpsimd.memset(ct.ap(), v)
            nc.const_aps.aps[(mybir.dt.float32, v)] = ct.ap()
    if need_consts:
        nc.all_engine_barrier()

    dqN_in = nc.declare_dram_parameter("dqN", [P, 2 * WN], mybir.dt.float32,
                                       isOutput=False)
    dqF_in = nc.declare_dram_parameter("dqF", [P, 2 * WF], mybir.dt.float32,
                                       isOutput=False)
    qicN_in = nc.declare_dram_parameter("qicN", [P, APP], mybir.dt.float32,
                                        isOutput=False)
    qicF_in = nc.declare_dram_parameter("qicF", [P, APP], mybir.dt.float32,
                                        isOutput=False)
    eN_out = nc.declare_dram_parameter("EN", [P, APP], mybir.dt.float32,
                                       isOutput=True)
    eF_out = nc.declare_dram_parameter("EF", [P, APP], mybir.dt.float32,
                                       isOutput=True)

    AL = mybir.AluOpType
    AF = mybir.ActivationFunctionType

    groupsN = _groups(KsN)
    groupsF = _groups(KsF)

    def _merged_runs(Ks, grp):
        runs = []
        for c in grp:
            if runs and Ks[c] == runs[-1][1]:
                runs[-1][2] += 1
            else:
                runs.append([c, Ks[c], 1])
        return runs

    with tile.TileContext(nc, num_cores=N_CORES) as tc:
        with tc.tile_pool(name="io", bufs=io_bufs) as io, \
             tc.tile_pool(name="tmp", bufs=tmp_bufs) as tp, \
             tc.tile_pool(name="keep", bufs=len(groupsN)) as kp, \
             tc.tile_pool(name="accp", bufs=1) as ap_pool, \
             tc.tile_pool(name="qicp", bufs=1) as qp_pool:
            qicN = qp_pool.tile([P, APP], mybir.dt.float32, tag="qicN")
            qicF = qp_pool.tile([P, APP], mybir.dt.float32, tag="qicF")
            nc.scalar.dma_start(qicN[:], qicN_in[:])
            nc.scalar.dma_start(qicF[:], qicF_in[:])
            accN = ap_pool.tile([P, APP], mybir.dt.float32, tag="accN")
            accF = ap_pool.tile([P, APP], mybir.dt.float32, tag="accF")

            def body():
                # ---- phase 1: near geometry (sqrt_and_others table set) ----
                ph1 = []  # per near group: (p, num, Qt, width, chunks)
                fence_srcs = []
                for grp in groupsN:
                    F = sum(CAT * KsN[c] for c in grp)
                    o = 2 * int(offsN[grp[0]])
                    Dt = io.tile([P, F], mybir.dt.float32, tag="DQF")
                    Qt = kp.tile([P, F], mybir.dt.float32, tag="QN")
                    if tiny_dma:
                        nc.sync.dma_start(Dt[:, :64], dqN_in[:, o:o + 64])
                        nc.gpsimd.dma_start(Qt[:, :64],
                                            dqN_in[:, o + F:o + F + 64])
                    else:
                        nc.sync.dma_start(Dt[:], dqN_in[:, o:o + F])
                        nc.gpsimd.dma_start(Qt[:], dqN_in[:, o + F:o + 2 * F])
                    if dma_only:
                        ph1.append((None, None, Qt, F, grp))
                        continue
                    D = Dt[:]
                    t = tp.tile([P, F], mybir.dt.float32, tag="t")
                    z = tp.tile([P, F], mybir.dt.float32, tag="z")
                    dm = tp.tile([P, F], mybir.dt.float32, tag="dm")
                    p = kp.tile([P, F], mybir.dt.float32, tag="p")
                    num = kp.tile([P, F], mybir.dt.float32, tag="num")

                    nc.scalar.activation(t[:], D, AF.Square)
                    if p_on_act:
                        # d*s = sqrt((d^2+0.5)^2 - 0.25)
                        nc.scalar.activation(p[:], t[:], AF.Square, bias=0.5)
                        nc.scalar.activation(p[:], p[:], AF.Sqrt, bias=-0.25)
                    # near stream has d < 5 strictly (padding d=1), so
                    # z = 1 - d/5 > 0 always - no relu needed
                    nc.vector.tensor_scalar(z[:], D, -2.0 / CUTOFF, 1.0,
                                            op0=AL.mult, op1=AL.add)
                    # s overwrites t in place (ACT in-place is safe)
                    nc.scalar.activation(t[:], t[:], AF.Sqrt, bias=1.0)
                    s = t
                    if not p_on_act:
                        nc.gpsimd.tensor_tensor(p[:], D, s[:], op=AL.mult)
                    if dm_on_gps:
                        nc.gpsimd.tensor_tensor(dm[:], D, s[:], op=AL.subtract)
                    else:
                        nc.vector.tensor_tensor(dm[:], D, s[:], op=AL.subtract)
                    # g1 = f(z) * dm in one fused DVE pass (in place over z)
                    nc.vector._custom_dve(fmul, out=z[:], in0=z[:], in1=dm[:],
                                          s0=SQ6, s1=C15, imm2=0.625)
                    nc.vector.tensor_tensor(num[:], z[:], s[:], op=AL.add)
                    ph1.append((p, num, Qt, F, grp))
                    fence_srcs.append(s)

                # ---- phase 2: seeds + Newton + scatter (natural_log_exp) ----
                # Fence: a [P,1] exact-0.0 tile whose producer chain reads one
                # column of every phase-1 Sqrt output.  Used as the bias AP of
                # every Ln, it forces the scheduler to keep ALL sqrt-set ACT
                # ops before ALL natural_log-set ops (the engine runs in
                # order, so interleaving would re-load tables ~2.7us a pop).
                fence = None
                if not dma_only:
                    for s_t in fence_srcs:
                        fence_new = tp.tile([P, 1], mybir.dt.float32,
                                            tag="fence")
                        nc.scalar.activation(
                            fence_new[:], s_t[:, 0:1],
                            AF.Identity, scale=0.0,
                            bias=fence[:] if fence is not None else 0.0)
                        fence = fence_new
                for p, num, Qt, F, grp in ph1:
                    if dma_only:
                        loc = 0
                        for c in grp:
                            K = KsN[c]
                            Fc = CAT * K
                            nc.vector.tensor_reduce(
                                accN[:, c * CAT:(c + 1) * CAT],
                                Qt[:, loc:loc + Fc].rearrange(
                                    "p (a k) -> p a k", k=K),
                                axis=mybir.AxisListType.X,
                                op=AL.add,
                            )
                            loc += Fc
                        continue
                    y = tp.tile([P, F], mybir.dt.float32, tag="y")
                    Yt = tp.tile([P, F], mybir.dt.float32, tag="Y")
                    if seed_dve:
                        nc.vector.reciprocal_approx_fast(y[:], p[:])
                    else:
                        nc.scalar.activation(y[:], p[:], AF.Ln,
                                             bias=fence[:] if fence is not None
                                             else 0.0)
                        nc.scalar.activation(y[:], y[:], AF.Exp, scale=-1.0)
                    # ir = double-Newton 1/p, in place over y
                    nc.vector._custom_dve(nr2, out=y[:],
                                          in0=p[:], in1=y[:], s0=2.0)
                    if yn_on_gps:
                        nc.gpsimd.tensor_tensor(Yt[:], y[:], Qt[:], op=AL.mult)
                    else:
                        nc.vector.tensor_tensor(Yt[:], y[:], Qt[:], op=AL.mult)
                    nc.vector.tensor_tensor(Yt[:], num[:], Yt[:], op=AL.mult)
                    loc = 0
                    for c0, K, nch in _merged_runs(KsN, grp):
                        Fr = CAT * K * nch
                        nc.vector.tensor_reduce(
                            accN[:, c0 * CAT:c0 * CAT + nch * CAT],
                            Yt[:, loc:loc + Fr].rearrange(
                                "p (a k) -> p a k", k=K),
                            axis=mybir.AxisListType.X,
                            op=AL.add,
                        )
                        loc += Fr

                for grp in groupsF:
                    F = sum(CAT * KsF[c] for c in grp)
                    o = 2 * int(offsF[grp[0]])
                    DQ = io.tile([P, 2 * F], mybir.dt.float32, tag="DQF")
                    nc.sync.dma_start(DQ[:], dqF_in[:, o:o + 2 * F])
                    Qt = DQ[:, F:]
                    D = DQ[:, :F]
                    if dma_only:
                        loc = 0
                        for c in grp:
                            K = KsF[c]
                            Fc = CAT * K
                            nc.vector.tensor_reduce(
                                accF[:, c * CAT:(c + 1) * CAT],
                                Qt[:, loc:loc + Fc].rearrange(
                                    "p (a k) -> p a k", k=K),
                                axis=mybir.AxisListType.X,
                                op=AL.add,
                            )
                            loc += Fc
                        continue
                    y = tp.tile([P, F], mybir.dt.float32, tag="y")
                    Yt = tp.tile([P, F], mybir.dt.float32, tag="Y")
                    if seed_dve:
                        nc.vector.reciprocal_approx_fast(y[:], D)
                    else:
                        nc.scalar.activation(y[:], D, AF.Ln,
                                             bias=fence[:] if fence is not None
                                             else 0.0)
                        nc.scalar.activation(y[:], y[:], AF.Exp, scale=-1.0)
                    nc.vector._custom_dve(nr2, out=y[:],
                                          in0=D, in1=y[:], s0=2.0)
                    if yf_on_gps:
                        nc.gpsimd.tensor_tensor(Yt[:], y[:], Qt, op=AL.mult)
                    else:
                        nc.vector.tensor_tensor(Yt[:], y[:], Qt, op=AL.mult)
                    loc = 0
                    for c0, K, nch in _merged_runs(KsF, grp):
                        Fr = CAT * K * nch
                        nc.vector.tensor_reduce(
                            accF[:, c0 * CAT:c0 * CAT + nch * CAT],
                            Yt[:, loc:loc + Fr].rearrange(
                                "p (a k) -> p a k", k=K),
                            axis=mybir.AxisListType.X,
                            op=AL.add,
                        )
                        loc += Fr

                # E = acc * qic (qic pre-scaled by +0.5*qi_c on host)
                nc.vector.tensor_tensor(accN[:], accN[:], qicN[:], op=AL.mult)
                nc.vector.tensor_tensor(accF[:], accF[:], qicF[:], op=AL.mult)
                nc.scalar.dma_start(eN_out[:], accN[:])
                nc.scalar.dma_start(eF_out[:], accF[:])

            if reps == 1:
                body()
            else:
                with tc.For_i(0, reps):
                    for _ in range(body_mult):
                        body()
    # populate .instr bytes for InstISA subclasses (custom DVE ops); without
    # this walrus fails with "ISA wrong length"
    mybir.codegen_inst_isa_subclasses(nc)
    return nc


class _Runner:
    """Compile once; keep a reusable jitted SPMD callable."""

    def __init__(self, nc):
        import jax
        from jax.sharding import Mesh, PartitionSpec, NamedSharding
        from jax.experimental.shard_map import shard_map
        import concourse.mybir as mybir
        import concourse.bass2jax as b2j
        b2j.install_neuronx_cc_hook()
        self.jax = jax
        in_names, out_names, out_avals, zero_outs = [], [], [], []
        pname = nc.partition_id_tensor.name if nc.partition_id_tensor else None
        for alloc in nc.m.functions[0].allocations:
            if not isinstance(alloc, mybir.MemoryLocationSet):
                continue
            name = alloc.memorylocations[0].name
            if alloc.kind == "ExternalInput":
                if name != pname:
                    in_names.append(name)
            elif alloc.kind == "ExternalOutput":
                shape = tuple(alloc.tensor_shape)
                dtype = mybir.dt.np(alloc.dtype)
                out_names.append(name)
                out_avals.append(jax.core.ShapedArray(shape, dtype))
                zero_outs.append(np.zeros(shape, dtype))
        self.in_names, self.out_names = in_names, out_names
        self.out_avals, self.zero_outs = out_avals, zero_outs
        all_in = list(in_names) + list(out_names) + ([pname] if pname else [])

        def _body(*args):
            operands = list(args)
            if pname is not None:
                operands.append(b2j.partition_id_tensor())
            return tuple(b2j._bass_exec_p.bind(
                *operands,
                out_avals=tuple(out_avals),
                in_names=tuple(all_in),
                out_names=tuple(out_names),
                lowering_input_output_aliases=(),
                sim_require_finite=True,
                sim_require_nnan=True,
                nc=nc,
            ))

        devices = jax.devices()[:N_CORES]
        mesh = Mesh(np.asarray(devices), ("core",))
        n_in = len(in_names) + len(zero_outs)
        self.fn = jax.jit(
            shard_map(_body, mesh=mesh,
                      in_specs=(PartitionSpec("core"),) * n_in,
                      out_specs=(PartitionSpec("core"),) * len(out_names),
                      check_rep=False),
            keep_unused=True,
        )
        self.sharding = NamedSharding(mesh, PartitionSpec("core"))

    def put_inputs(self, in_maps, device_resident=False):
        args = []
        for name in self.in_names:
            cat = np.concatenate([np.asarray(m[name]) for m in in_maps], axis=0)
            args.append(cat)
        for z in self.zero_outs:
            args.append(np.zeros((N_CORES * z.shape[0], *z.shape[1:]), z.dtype))
        if device_resident:
            try:
                jax = self.jax
                devices = list(self.sharding.mesh.devices.reshape(-1))
                put = []
                for a in args:
                    per = a.shape[0] // N_CORES
                    shards = [
                        jax.device_put(a[c * per:(c + 1) * per], devices[c])
                        for c in range(N_CORES)
                    ]
                    put.append(jax.make_array_from_single_device_arrays(
                        a.shape, self.sharding, shards))
                jax.block_until_ready(put)
                args = put
            except Exception:
                pass
        return args

    def run(self, args):
        outs = self.fn(*args)
        self.jax.block_until_ready(outs)
        return outs

    def results(self, outs):
        res = []
        for c in range(N_CORES):
            res.append({
                name: np.asarray(outs[i]).reshape(N_CORES, *self.out_avals[i].shape)[c]
                for i, name in enumerate(self.out_names)
            })
        return res


def _get_runner(KsN, KsF, reps=1, body_mult=1, **bk):
    key = (tuple(KsN), tuple(KsF), reps, body_mult, tuple(sorted(bk.items())))
    if key not in _RUNNER_CACHE:
        _install_compat()
        _RUNNER_CACHE[key] = _Runner(
            _build_nc(tuple(KsN), tuple(KsF), reps, body_mult, **bk))
    return _RUNNER_CACHE[key]


# ---------------------------------------------------------------------------
# host-side shard construction: one degree-bucketed CSR layout per stream
# ---------------------------------------------------------------------------
def _stream_layout(ii_sub, d_sub, qj_sub):
    """Build the [N_CORES*P, 2W] interleaved (d, qj) grid + per-atom output
    position maps for one edge subset (edges targeting atom ii_sub)."""
    counts = np.bincount(ii_sub, minlength=N_ATOMS)
    a_order = np.argsort(-counts, kind="stable")
    degs = counts[a_order]
    n_chunks = APP // CAT
    cg = N_CORES * P * CAT
    Ks = tuple(-2 * (-int(degs[c * cg:(c + 1) * cg].max()) // 2)
               for c in range(n_chunks))  # ceil to even: merges reduce instrs
    W = sum(CAT * k for k in Ks)
    offs_c = np.cumsum([0] + [CAT * k for k in Ks])

    groups = _groups(Ks)
    c0_of_c = np.empty(n_chunks, np.int64)
    fg_of_c = np.empty(n_chunks, np.int64)
    for g in groups:
        fg = sum(CAT * Ks[c] for c in g)
        for c in g:
            c0_of_c[c] = g[0]
            fg_of_c[c] = fg
    rank = np.arange(N_ATOMS, dtype=np.int64)
    core = rank % N_CORES
    r = rank // N_CORES
    c_of = r // (P * CAT)
    w = r % (P * CAT)
    p_of = w % P
    j_of = w // P
    row = core * P + p_of
    colE = c_of * CAT + j_of
    kc = np.asarray(Ks, np.int64)[c_of]
    dcol = offs_c[c_of] + offs_c[c0_of_c[c_of]]
    based = row * 2 * W + dcol + j_of * kc

    row_of = np.empty(N_ATOMS, np.int64)
    colE_of = np.empty(N_ATOMS, np.int64)
    based_of = np.empty(N_ATOMS, np.int64)
    fc_of = np.empty(N_ATOMS, np.int64)
    row_of[a_order] = row
    colE_of[a_order] = colE
    based_of[a_order] = based
    fc_of[a_order] = fg_of_c[c_of]

    e_order = np.argsort(ii_sub, kind="stable")
    i_s = ii_sub[e_order]
    csr = np.zeros(N_ATOMS, np.int64)
    np.cumsum(counts[:-1], out=csr[1:])
    slot = np.arange(len(ii_sub), dtype=np.int64) - csr[i_s]
    pos_d = based_of[i_s] + slot
    pos_q = pos_d + fc_of[i_s]

    dq = np.zeros((N_CORES * P, 2 * W), np.float32)
    for g in groups:
        o = 2 * int(offs_c[g[0]])
        fg = sum(CAT * Ks[c] for c in g)
        dq[:, o:o + fg] = 1.0            # d padding (avoid ln(0))
    dq = dq.reshape(-1)
    dq[pos_d] = d_sub[e_order]
    dq[pos_q] = qj_sub[e_order]
    return {
        "dq": dq.reshape(N_CORES * P, 2 * W),
        "Ks": Ks,
        "row_of": row_of,
        "colE_of": colE_of,
    }


def _prep(qi, edge_dist, edge_index, q_ref, N, atom_mol_batch):
    qi = np.asarray(qi, np.float32)
    edge_dist = np.asarray(edge_dist, np.float32)
    ii = np.asarray(edge_index[0], np.int64)
    jj = np.asarray(edge_index[1], np.int64)
    # charge-neutrality correction (index-driven segment sum over atoms)
    q_mol = np.bincount(np.asarray(atom_mol_batch, np.int64), weights=qi,
                        minlength=N_MOL).astype(np.float32)
    corr = (q_mol - np.asarray(q_ref, np.float32)) / np.asarray(N, np.float32)
    qi_c = qi - corr[np.asarray(atom_mol_batch, np.int64)]
    qj_c = qi_c[jj]

    near = edge_dist < (CUTOFF / 2.0)
    farm = ~near
    LN = _stream_layout(ii[near], edge_dist[near], qj_c[near])
    LF = _stream_layout(ii[farm], edge_dist[farm], qj_c[farm])

    # qic grids pre-scaled by +0.5*qi_c (0.5 = double-counting factor; the
    # NR reciprocal is positive, unlike v1's negated form)
    qic = qi_c * np.float32(0.5)
    qicN = np.zeros((N_CORES * P, APP), np.float32)
    qicF = np.zeros((N_CORES * P, APP), np.float32)
    qicN[LN["row_of"], LN["colE_of"]] = qic
    qicF[LF["row_of"], LF["colE_of"]] = qic
    return {
        "dqN": LN["dq"], "dqF": LF["dq"],
        "KsN": LN["Ks"], "KsF": LF["Ks"],
        "qicN": qicN, "qicF": qicF,
        "rowN": LN["row_of"], "colN": LN["colE_of"],
        "rowF": LF["row_of"], "colF": LF["colE_of"],
    }


def _shard_maps(prep):
    in_maps = []
    for c in range(N_CORES):
        rs = slice(c * P, (c + 1) * P)
        in_maps.append({
            "dqN": prep["dqN"][rs],
            "dqF": prep["dqF"][rs],
            "qicN": prep["qicN"][rs],
            "qicF": prep["qicF"][rs],
        })
    return in_maps


def _unshard(prep, res):
    eN = np.concatenate([r["EN"].reshape(P, APP) for r in res], axis=0)
    eF = np.concatenate([r["EF"].reshape(P, APP) for r in res], axis=0)
    out = eN[prep["rowN"], prep["colN"]] + eF[prep["rowF"], prep["colF"]]
    return out.astype(np.float32)


def kernel(qi, edge_dist, edge_index, q_ref, N, atom_mol_batch):
    prep = _prep(qi, edge_dist, edge_index, q_ref, N, atom_mol_batch)
    runner = _get_runner(prep["KsN"], prep["KsF"])
    maps = _shard_maps(prep)
    # Deterministic computation: rerun until two consecutive results agree
    # bit-exactly (the axon tunnel occasionally corrupts a dispatch).
    prev = None
    for _ in range(5):
        args = runner.put_inputs(maps)
        res = runner.results(runner.run(args))
        out = _unshard(prep, res)
        if prev is not None and np.array_equal(out, prev):
            return out
        prev = out
    return out


# revision 5
# speedup vs baseline: 1.0759x; 1.0759x over previous
"""CoulombLayer Trainium2 kernel v2 (8 NeuronCores, SPMD via bass).

Strategy vs v1 (baseline):
  * Edges are split on host into NEAR (d < 5, inside the smooth-cutoff
    transition of f(2d) with cutoff=10) and FAR (d >= 5, where the PhysNet
    cutoff f is exactly 0 so chi(d) = 1/d exactly).  69% of edges are far
    and need only a reciprocal and a multiply - no sqrt / smoothstep math.
  * Each stream gets its own vertex-cut CSR layout (degree-sorted atoms,
    dealt round-robin to cores, chunked with shared slot width K_c), its own
    accumulator grid and its own output; the host adds the two grids while
    unsharding (index-driven gather it already does anyway).
  * Reciprocals run as: ACT seed y = exp(-ln x) (natural_log_exp table set)
    + ONE fused Newton step on DVE via the custom-DVE op
    RECIPROCAL_APPROX_NR: ir = (2 - x*y)*y  (1 instruction instead of 2).
  * The near smoothstep g = f(z)*dm, f = z^3*((sqrt6 z - c)^2 + 0.625), is
    ONE authored custom-DVE instruction (F_SMOOTH_MUL, 8 ALU stages), with
    z = relu(1 - d/5) from ACT and dm = d - s from GPSIMD.
  * Engine balance per core (est): DVE ~52us, ACT ~50us, GPSIMD ~49us,
    DMA ~48us (fp32 memory floor).  Everything fp32: the 2e-2 max-rel-err
    gate is dominated by atoms with catastrophic term cancellation, which
    demands ~1e-7 per-term accuracy - no fp16/bf16 and no approximations
    beyond few-ulp.
  * ACT table sets: phase 1 (near s/z/p math) uses sqrt_and_others; phase 2
    (all seeds) uses natural_log_exp_and_others -> exactly 2 table loads per
    execution.
"""

import json as _json
import numpy as np

N_CORES = 8
N_ATOMS = 500_000
N_MOL = 5_000
N_EDGES = 16_000_000
CUTOFF = 10.0
P = 125                 # SBUF partitions used (125 * 500 = 62500 atoms/core)
APP = 500               # atoms per partition
CAT = 25                # atoms per compute tile (per partition)
GCAP = 2050             # max elements per grouped elementwise op

SQ6 = 6.0 ** 0.5
C15 = 15.0 / (2.0 * SQ6)

_RUNNER_CACHE = {}


# ---------------------------------------------------------------------------
# authored custom DVE ops (registered into concourse.dve_ops at first use):
#   F_SMOOTH_MUL_ANT: g = f(z)*dm, f = ((sqrt6*z - c)^2 + 0.625)*z^3 (the
#     PhysNet quintic smoothstep in z = relu(1 - d/5) form). 8 ALU stages,
#     one DVE pass instead of TT+STT+TT plus two ACT squares.
#   RECIP_NR2_ANT: two fused Newton steps y <- y*(2 - x*y) (6 stages). From
#     the ~1.2e-3 ACT exp(-ln x) seed this converges to the fp32 rounding
#     floor (~1.4e-7) in ONE DVE pass - the single-NR version left ~1e-6
#     per-term error, which the max-rel-err gate amplifies via cancellation.
# ---------------------------------------------------------------------------
_CUSTOM_OPS = None


def _get_custom_ops():
    global _CUSTOM_OPS
    if _CUSTOM_OPS is not None:
        return _CUSTOM_OPS
    import concourse.dve_ops as dve_ops
    from concourse.dve_spec import Spec, Src0, Src1, C0, C1, C2, lower, sq
    from concourse.dve_uop import DveOpSpec

    def _register(name, spec):
        if name in dve_ops._SUB_OPCODE_FOR_NAME:
            return next(op for op in dve_ops.OPS if op.name == name)
        row = max(dve_ops._SUB_OPCODE_FOR_NAME.values()) + 1
        assert row < 0x20
        shas = {}
        for ver in ("v3", "v4"):
            uops = lower(spec, ver=ver)
            shas[ver] = DveOpSpec(name=name, opcode=row, uops=uops,
                                  rd1_en=True).sha(ver)
        op = dve_ops.DveOp(name=name, spec=spec, subdim=False, uops_sha=shas)
        dve_ops._SUB_OPCODE_FOR_NAME[name] = row
        dve_ops.OPS.append(op)
        dve_ops.CUSTOM_DVE_SPECS[name] = spec
        return op

    def _fmul_ref(in0, in1, s0, s1, imm2):
        z = in0.astype(np.float32)
        f = ((z * s0 - s1) ** 2 + imm2) * z * z * z
        return (f * in1).astype(np.float32)

    fmul = _register("F_SMOOTH_MUL_ANT", Spec(
        body=(sq(Src0 * C0 - C1) + C2) * sq(Src0) * Src0 * Src1,
        reference=_fmul_ref,
    ))

    def _nr2_ref(in0, in1, s0, s1, imm2):
        t = ((s0 - in0 * in1) * in1).astype(np.float32)
        return ((s0 - in0 * t) * t).astype(np.float32)

    _y1 = (C0 - Src0 * Src1) * Src1
    nr2 = _register("RECIP_NR2_ANT", Spec(
        body=(C0 - Src0 * _y1) * _y1,
        reference=_nr2_ref,
    ))
    _CUSTOM_OPS = (fmul, nr2)
    return _CUSTOM_OPS


# ---------------------------------------------------------------------------
# walrus compat: this build rejects >1 sync-wait per instruction.  Split
# overflow waits onto NoOps inserted immediately before, same engine/block.
# ---------------------------------------------------------------------------
def _fix_bir_json(bir_json):
    m = _json.loads(bir_json)
    for fn in m.get("functions", []):
        for blk in fn.get("blocks", []):
            out = []
            for inst in blk.get("instructions", []):
                si = inst.get("sync_info")
                waits = (si or {}).get("on_wait", [])
                if len(waits) > 1:
                    for k, w in enumerate(waits[:-1]):
                        out.append({
                            "debug": inst.get("debug", 0),
                            "engine": inst["engine"],
                            "ins": [],
                            "name": f"{inst['name']}-sw{k}",
                            "opcode": "NoOp",
                            "outs": [],
                            "sync_info": {"on_update": [], "on_wait": [w]},
                        })
                    si["on_wait"] = [waits[-1]]
                out.append(inst)
            blk["instructions"] = out
    return _json.dumps(m).encode()


_PATCHED = False


def _install_compat():
    global _PATCHED
    if _PATCHED:
        return
    _PATCHED = True
    import concourse.bass_utils as bu
    import concourse.bass2jax as b2j
    orig = bu.compile_bir_kernel

    def patched(bir_json, tmpdir, neff_name="file.neff"):
        return orig(_fix_bir_json(bir_json), tmpdir, neff_name)

    bu.compile_bir_kernel = patched
    b2j.compile_bir_kernel = patched


def _groups(Ks):
    """Greedy-pack consecutive chunks into groups of total width <= GCAP."""
    out = []
    cur = [0]
    w = CAT * Ks[0]
    for c in range(1, len(Ks)):
        f = CAT * Ks[c]
        if w + f <= GCAP:
            cur.append(c)
            w += f
        else:
            out.append(cur)
            cur = [c]
            w = f
    out.append(cur)
    return out


# ---------------------------------------------------------------------------
# device program
# ---------------------------------------------------------------------------
def _build_nc(KsN, KsF, reps=1, body_mult=1,
              p_on_act=False, dm_on_gps=True, yn_on_gps=True, yf_on_gps=True,
              seed_dve=False, dma_only=False, tiny_dma=False,
              io_bufs=2, tmp_bufs=2):
    """Two-stream device program for one core.

    KsN/KsF: per-chunk slot widths for the near / far CSR layouts."""
    import concourse.bass as bass
    import concourse.mybir as mybir
    import concourse.tile as tile

    fmul, nr2 = _get_custom_ops()

    WN = sum(CAT * k for k in KsN)
    WF = sum(CAT * k for k in KsF)
    offsN = np.cumsum([0] + [CAT * k for k in KsN])
    offsF = np.cumsum([0] + [CAT * k for k in KsF])
    nc = bass.Bass()

    # const-AP pool entries for non-Copy activation biases (only 0.0/1.0 are
    # pre-registered)
    need_consts = [-0.25, 0.5] if p_on_act else []
    for v in need_consts:
        if (mybir.dt.float32, v) not in nc.const_aps.aps:
            ct = nc.alloc_sbuf_tensor(f"const-float32-{v}", [128, 1],
                                      mybir.dt.float32)
            nc.gpsimd.memset(ct.ap(), v)
            nc.const_aps.aps[(mybir.dt.float32, v)] = ct.ap()
    if need_consts:
        nc.all_engine_barrier()

    dqN_in = nc.declare_dram_parameter("dqN", [P, 2 * WN], mybir.dt.float32,
                                       isOutput=False)
    dqF_in = nc.declare_dram_parameter("dqF", [P, 2 * WF], mybir.dt.float32,
                                       isOutput=False)
    qicN_in = nc.declare_dram_parameter("qicN", [P, APP], mybir.dt.float32,
                                        isOutput=False)
    qicF_in = nc.declare_dram_parameter("qicF", [P, APP], mybir.dt.float32,
                                        isOutput=False)
    eN_out = nc.declare_dram_parameter("EN", [P, APP], mybir.dt.float32,
                                       isOutput=True)
    eF_out = nc.declare_dram_parameter("EF", [P, APP], mybir.dt.float32,
                                       isOutput=True)

    AL = mybir.AluOpType
    AF = mybir.ActivationFunctionType

    groupsN = _groups(KsN)
    groupsF = _groups(KsF)

    def _merged_runs(Ks, grp):
        runs = []
        for c in grp:
            if runs and Ks[c] == runs[-1][1]:
                runs[-1][2] += 1
            else:
                runs.append([c, Ks[c], 1])
        return runs

    with tile.TileContext(nc, num_cores=N_CORES) as tc:
        with tc.tile_pool(name="io", bufs=io_bufs) as io, \
             tc.tile_pool(name="tmp", bufs=tmp_bufs) as tp, \
             tc.tile_pool(name="keep", bufs=len(groupsN)) as kp, \
             tc.tile_pool(name="accp", bufs=1) as ap_pool, \
             tc.tile_pool(name="qicp", bufs=1) as qp_pool:
            qicN = qp_pool.tile([P, APP], mybir.dt.float32, tag="qicN")
            qicF = qp_pool.tile([P, APP], mybir.dt.float32, tag="qicF")
            nc.scalar.dma_start(qicN[:], qicN_in[:])
            nc.scalar.dma_start(qicF[:], qicF_in[:])
            accN = ap_pool.tile([P, APP], mybir.dt.float32, tag="accN")
            accF = ap_pool.tile([P, APP], mybir.dt.float32, tag="accF")

            def body():
                # ---- phase 1: near geometry (sqrt_and_others table set) ----
                ph1 = []  # per near group: (p, num, Qt, width, chunks)
                fence_srcs = []
                for grp in groupsN:
                    F = sum(CAT * KsN[c] for c in grp)
                    o = 2 * int(offsN[grp[0]])
                    Dt = io.tile([P, F], mybir.dt.float32, tag="D")
                    Qt = kp.tile([P, F], mybir.dt.float32, tag="QN")
                    if tiny_dma:
                        nc.sync.dma_start(Dt[:, :64], dqN_in[:, o:o + 64])
                        nc.gpsimd.dma_start(Qt[:, :64],
                                            dqN_in[:, o + F:o + F + 64])
                    else:
                        nc.sync.dma_start(Dt[:], dqN_in[:, o:o + F])
                        nc.gpsimd.dma_start(Qt[:], dqN_in[:, o + F:o + 2 * F])
                    if dma_only:
                        ph1.append((None, None, Qt, F, grp))
                        continue
                    D = Dt[:]
                    t = tp.tile([P, F], mybir.dt.float32, tag="t")
                    z = tp.tile([P, F], mybir.dt.float32, tag="z")
                    dm = tp.tile([P, F], mybir.dt.float32, tag="dm")
                    p = kp.tile([P, F], mybir.dt.float32, tag="p")
                    num = kp.tile([P, F], mybir.dt.float32, tag="num")

                    nc.scalar.activation(t[:], D, AF.Square)
                    if p_on_act:
                        # d*s = sqrt((d^2+0.5)^2 - 0.25)
                        nc.scalar.activation(p[:], t[:], AF.Square, bias=0.5)
                        nc.scalar.activation(p[:], p[:], AF.Sqrt, bias=-0.25)
                    # near stream has d < 5 strictly (padding d=1), so
                    # z = 1 - d/5 > 0 always - no relu needed
                    nc.vector.tensor_scalar(z[:], D, -2.0 / CUTOFF, 1.0,
                                            op0=AL.mult, op1=AL.add)
                    # s overwrites t in place (ACT in-place is safe)
                    nc.scalar.activation(t[:], t[:], AF.Sqrt, bias=1.0)
                    s = t
                    if not p_on_act:
                        nc.gpsimd.tensor_tensor(p[:], D, s[:], op=AL.mult)
                    if dm_on_gps:
                        nc.gpsimd.tensor_tensor(dm[:], D, s[:], op=AL.subtract)
                    else:
                        nc.vector.tensor_tensor(dm[:], D, s[:], op=AL.subtract)
                    # g1 = f(z) * dm in one fused DVE pass (in place over z)
                    nc.vector._custom_dve(fmul, out=z[:], in0=z[:], in1=dm[:],
                                          s0=SQ6, s1=C15, imm2=0.625)
                    nc.vector.tensor_tensor(num[:], z[:], s[:], op=AL.add)
                    ph1.append((p, num, Qt, F, grp))
                    fence_srcs.append(s)

                # ---- phase 2: seeds + Newton + scatter (natural_log_exp) ----
                # Fence: a [P,1] exact-0.0 tile whose producer chain reads one
                # column of every phase-1 Sqrt output.  Used as the bias AP of
                # every Ln, it forces the scheduler to keep ALL sqrt-set ACT
                # ops before ALL natural_log-set ops (the engine runs in
                # order, so interleaving would re-load tables ~2.7us a pop).
                fence = None
                if not dma_only:
                    for s_t in fence_srcs:
                        fence_new = tp.tile([P, 1], mybir.dt.float32,
                                            tag="fence")
                        nc.scalar.activation(
                            fence_new[:], s_t[:, 0:1],
                            AF.Identity, scale=0.0,
                            bias=fence[:] if fence is not None else 0.0)
                        fence = fence_new
                for p, num, Qt, F, grp in ph1:
                    if dma_only:
                        loc = 0
                        for c in grp:
                            K = KsN[c]
                            Fc = CAT * K
                            nc.vector.tensor_reduce(
                                accN[:, c * CAT:(c + 1) * CAT],
                                Qt[:, loc:loc + Fc].rearrange(
                                    "p (a k) -> p a k", k=K),
                                axis=mybir.AxisListType.X,
                                op=AL.add,
                            )
                            loc += Fc
                        continue
                    y = tp.tile([P, F], mybir.dt.float32, tag="y")
                    Yt = tp.tile([P, F], mybir.dt.float32, tag="Y")
                    if seed_dve:
                        nc.vector.reciprocal_approx_fast(y[:], p[:])
                    else:
                        nc.scalar.activation(y[:], p[:], AF.Ln,
                                             bias=fence[:] if fence is not None
                                             else 0.0)
                        nc.scalar.activation(y[:], y[:], AF.Exp, scale=-1.0)
                    # ir = double-Newton 1/p, in place over y
                    nc.vector._custom_dve(nr2, out=y[:],
                                          in0=p[:], in1=y[:], s0=2.0)
                    if yn_on_gps:
                        nc.gpsimd.tensor_tensor(Yt[:], y[:], Qt[:], op=AL.mult)
                    else:
                        nc.vector.tensor_tensor(Yt[:], y[:], Qt[:], op=AL.mult)
                    nc.vector.tensor_tensor(Yt[:], num[:], Yt[:], op=AL.mult)
                    loc = 0
                    for c0, K, nch in _merged_runs(KsN, grp):
                        Fr = CAT * K * nch
                        nc.vector.tensor_reduce(
                            accN[:, c0 * CAT:c0 * CAT + nch * CAT],
                            Yt[:, loc:loc + Fr].rearrange(
                                "p (a k) -> p a k", k=K),
                            axis=mybir.AxisListType.X,
                            op=AL.add,
                        )
                        loc += Fr

                for grp in groupsF:
                    F = sum(CAT * KsF[c] for c in grp)
                    o = 2 * int(offsF[grp[0]])
                    Dt = io.tile([P, F], mybir.dt.float32, tag="D")
                    Qt = io.tile([P, F], mybir.dt.float32, tag="Q")
                    if tiny_dma:
                        nc.sync.dma_start(Dt[:, :64], dqF_in[:, o:o + 64])
                        nc.gpsimd.dma_start(Qt[:, :64],
                                            dqF_in[:, o + F:o + F + 64])
                    else:
                        nc.sync.dma_start(Dt[:], dqF_in[:, o:o + F])
                        nc.gpsimd.dma_start(Qt[:], dqF_in[:, o + F:o + 2 * F])
                    D = Dt[:]
                    if dma_only:
                        loc = 0
                        for c in grp:
                            K = KsF[c]
                            Fc = CAT * K
                            nc.vector.tensor_reduce(
                                accF[:, c * CAT:(c + 1) * CAT],
                                Qt[:, loc:loc + Fc].rearrange(
                                    "p (a k) -> p a k", k=K),
                                axis=mybir.AxisListType.X,
                                op=AL.add,
                            )
                            loc += Fc
                        continue
                    y = tp.tile([P, F], mybir.dt.float32, tag="y")
                    Yt = tp.tile([P, F], mybir.dt.float32, tag="Y")
                    if seed_dve:
                        nc.vector.reciprocal_approx_fast(y[:], D)
                    else:
                        nc.scalar.activation(y[:], D, AF.Ln,
                                             bias=fence[:] if fence is not None
                                             else 0.0)
                        nc.scalar.activation(y[:], y[:], AF.Exp, scale=-1.0)
                    nc.vector._custom_dve(nr2, out=y[:],
                                          in0=D, in1=y[:], s0=2.0)
                    if yf_on_gps:
                        nc.gpsimd.tensor_tensor(Yt[:], y[:], Qt[:], op=AL.mult)
                    else:
                        nc.vector.tensor_tensor(Yt[:], y[:], Qt[:], op=AL.mult)
                    loc = 0
                    for c0, K, nch in _merged_runs(KsF, grp):
                        Fr = CAT * K * nch
                        nc.vector.tensor_reduce(
                            accF[:, c0 * CAT:c0 * CAT + nch * CAT],
                            Yt[:, loc:loc + Fr].rearrange(
                                "p (a k) -> p a k", k=K),
                            axis=mybir.AxisListType.X,
                            op=AL.add,
                        )
                        loc += Fr

                # E = acc * qic (qic pre-scaled by +0.5*qi_c on host)
                nc.vector.tensor_tensor(accN[:], accN[:], qicN[:], op=AL.mult)
                nc.vector.tensor_tensor(accF[:], accF[:], qicF[:], op=AL.mult)
                nc.scalar.dma_start(eN_out[:], accN[:])
                nc.scalar.dma_start(eF_out[:], accF[:])

            if reps == 1:
                body()
            else:
                with tc.For_i(0, reps):
                    for _ in range(body_mult):
                        body()
    # populate .instr bytes for InstISA subclasses (custom DVE ops); without
    # this walrus fails with "ISA wrong length"
    mybir.codegen_inst_isa_subclasses(nc)
    return nc


class _Runner:
    """Compile once; keep a reusable jitted SPMD callable."""

    def __init__(self, nc):
        import jax
        from jax.sharding import Mesh, PartitionSpec, NamedSharding
        from jax.experimental.shard_map import shard_map
        import concourse.mybir as mybir
        import concourse.bass2jax as b2j
        b2j.install_neuronx_cc_hook()
        self.jax = jax
        in_names, out_names, out_avals, zero_outs = [], [], [], []
        pname = nc.partition_id_tensor.name if nc.partition_id_tensor else None
        for alloc in nc.m.functions[0].allocations:
            if not isinstance(alloc, mybir.MemoryLocationSet):
                continue
            name = alloc.memorylocations[0].name
            if alloc.kind == "ExternalInput":
                if name != pname:
                    in_names.append(name)
            elif alloc.kind == "ExternalOutput":
                shape = tuple(alloc.tensor_shape)
                dtype = mybir.dt.np(alloc.dtype)
                out_names.append(name)
                out_avals.append(jax.core.ShapedArray(shape, dtype))
                zero_outs.append(np.zeros(shape, dtype))
        self.in_names, self.out_names = in_names, out_names
        self.out_avals, self.zero_outs = out_avals, zero_outs
        all_in = list(in_names) + list(out_names) + ([pname] if pname else [])

        def _body(*args):
            operands = list(args)
            if pname is not None:
                operands.append(b2j.partition_id_tensor())
            return tuple(b2j._bass_exec_p.bind(
                *operands,
                out_avals=tuple(out_avals),
                in_names=tuple(all_in),
                out_names=tuple(out_names),
                lowering_input_output_aliases=(),
                sim_require_finite=True,
                sim_require_nnan=True,
                nc=nc,
            ))

        devices = jax.devices()[:N_CORES]
        mesh = Mesh(np.asarray(devices), ("core",))
        n_in = len(in_names) + len(zero_outs)
        self.fn = jax.jit(
            shard_map(_body, mesh=mesh,
                      in_specs=(PartitionSpec("core"),) * n_in,
                      out_specs=(PartitionSpec("core"),) * len(out_names),
                      check_rep=False),
            keep_unused=True,
        )
        self.sharding = NamedSharding(mesh, PartitionSpec("core"))

    def put_inputs(self, in_maps, device_resident=False):
        args = []
        for name in self.in_names:
            cat = np.concatenate([np.asarray(m[name]) for m in in_maps], axis=0)
            args.append(cat)
        for z in self.zero_outs:
            args.append(np.zeros((N_CORES * z.shape[0], *z.shape[1:]), z.dtype))
        if device_resident:
            try:
                jax = self.jax
                devices = list(self.sharding.mesh.devices.reshape(-1))
                put = []
                for a in args:
                    per = a.shape[0] // N_CORES
                    shards = [
                        jax.device_put(a[c * per:(c + 1) * per], devices[c])
                        for c in range(N_CORES)
                    ]
                    put.append(jax.make_array_from_single_device_arrays(
                        a.shape, self.sharding, shards))
                jax.block_until_ready(put)
                args = put
            except Exception:
                pass
        return args

    def run(self, args):
        outs = self.fn(*args)
        self.jax.block_until_ready(outs)
        return outs

    def results(self, outs):
        res = []
        for c in range(N_CORES):
            res.append({
                name: np.asarray(outs[i]).reshape(N_CORES, *self.out_avals[i].shape)[c]
                for i, name in enumerate(self.out_names)
            })
        return res


def _get_runner(KsN, KsF, reps=1, body_mult=1, **bk):
    key = (tuple(KsN), tuple(KsF), reps, body_mult, tuple(sorted(bk.items())))
    if key not in _RUNNER_CACHE:
        _install_compat()
        _RUNNER_CACHE[key] = _Runner(
            _build_nc(tuple(KsN), tuple(KsF), reps, body_mult, **bk))
    return _RUNNER_CACHE[key]


# ---------------------------------------------------------------------------
# host-side shard construction: one degree-bucketed CSR layout per stream
# ---------------------------------------------------------------------------
def _stream_layout(ii_sub, d_sub, qj_sub):
    """Build the [N_CORES*P, 2W] interleaved (d, qj) grid + per-atom output
    position maps for one edge subset (edges targeting atom ii_sub)."""
    counts = np.bincount(ii_sub, minlength=N_ATOMS)
    a_order = np.argsort(-counts, kind="stable")
    degs = counts[a_order]
    n_chunks = APP // CAT
    cg = N_CORES * P * CAT
    Ks = tuple(-2 * (-int(degs[c * cg:(c + 1) * cg].max()) // 2)
               for c in range(n_chunks))  # ceil to even: merges reduce instrs
    W = sum(CAT * k for k in Ks)
    offs_c = np.cumsum([0] + [CAT * k for k in Ks])

    groups = _groups(Ks)
    c0_of_c = np.empty(n_chunks, np.int64)
    fg_of_c = np.empty(n_chunks, np.int64)
    for g in groups:
        fg = sum(CAT * Ks[c] for c in g)
        for c in g:
            c0_of_c[c] = g[0]
            fg_of_c[c] = fg
    rank = np.arange(N_ATOMS, dtype=np.int64)
    core = rank % N_CORES
    r = rank // N_CORES
    c_of = r // (P * CAT)
    w = r % (P * CAT)
    p_of = w % P
    j_of = w // P
    row = core * P + p_of
    colE = c_of * CAT + j_of
    kc = np.asarray(Ks, np.int64)[c_of]
    dcol = offs_c[c_of] + offs_c[c0_of_c[c_of]]
    based = row * 2 * W + dcol + j_of * kc

    row_of = np.empty(N_ATOMS, np.int64)
    colE_of = np.empty(N_ATOMS, np.int64)
    based_of = np.empty(N_ATOMS, np.int64)
    fc_of = np.empty(N_ATOMS, np.int64)
    row_of[a_order] = row
    colE_of[a_order] = colE
    based_of[a_order] = based
    fc_of[a_order] = fg_of_c[c_of]

    e_order = np.argsort(ii_sub, kind="stable")
    i_s = ii_sub[e_order]
    csr = np.zeros(N_ATOMS, np.int64)
    np.cumsum(counts[:-1], out=csr[1:])
    slot = np.arange(len(ii_sub), dtype=np.int64) - csr[i_s]
    pos_d = based_of[i_s] + slot
    pos_q = pos_d + fc_of[i_s]

    dq = np.zeros((N_CORES * P, 2 * W), np.float32)
    for g in groups:
        o = 2 * int(offs_c[g[0]])
        fg = sum(CAT * Ks[c] for c in g)
        dq[:, o:o + fg] = 1.0            # d padding (avoid ln(0))
    dq = dq.reshape(-1)
    dq[pos_d] = d_sub[e_order]
    dq[pos_q] = qj_sub[e_order]
    return {
        "dq": dq.reshape(N_CORES * P, 2 * W),
        "Ks": Ks,
        "row_of": row_of,
        "colE_of": colE_of,
    }


def _prep(qi, edge_dist, edge_index, q_ref, N, atom_mol_batch):
    qi = np.asarray(qi, np.float32)
    edge_dist = np.asarray(edge_dist, np.float32)
    ii = np.asarray(edge_index[0], np.int64)
    jj = np.asarray(edge_index[1], np.int64)
    # charge-neutrality correction (index-driven segment sum over atoms)
    q_mol = np.bincount(np.asarray(atom_mol_batch, np.int64), weights=qi,
                        minlength=N_MOL).astype(np.float32)
    corr = (q_mol - np.asarray(q_ref, np.float32)) / np.asarray(N, np.float32)
    qi_c = qi - corr[np.asarray(atom_mol_batch, np.int64)]
    qj_c = qi_c[jj]

    near = edge_dist < (CUTOFF / 2.0)
    farm = ~near
    LN = _stream_layout(ii[near], edge_dist[near], qj_c[near])
    LF = _stream_layout(ii[farm], edge_dist[farm], qj_c[farm])

    # qic grids pre-scaled by +0.5*qi_c (0.5 = double-counting factor; the
    # NR reciprocal is positive, unlike v1's negated form)
    qic = qi_c * np.float32(0.5)
    qicN = np.zeros((N_CORES * P, APP), np.float32)
    qicF = np.zeros((N_CORES * P, APP), np.float32)
    qicN[LN["row_of"], LN["colE_of"]] = qic
    qicF[LF["row_of"], LF["colE_of"]] = qic
    return {
        "dqN": LN["dq"], "dqF": LF["dq"],
        "KsN": LN["Ks"], "KsF": LF["Ks"],
        "qicN": qicN, "qicF": qicF,
        "rowN": LN["row_of"], "colN": LN["colE_of"],
        "rowF": LF["row_of"], "colF": LF["colE_of"],
    }


def _shard_maps(prep):
    in_maps = []
    for c in range(N_CORES):
        rs = slice(c * P, (c + 1) * P)
        in_maps.append({
            "dqN": prep["dqN"][rs],
            "dqF": prep["dqF"][rs],
            "qicN": prep["qicN"][rs],
            "qicF": prep["qicF"][rs],
        })
    return in_maps


def _unshard(prep, res):
    eN = np.concatenate([r["EN"].reshape(P, APP) for r in res], axis=0)
    eF = np.concatenate([r["EF"].reshape(P, APP) for r in res], axis=0)
    out = eN[prep["rowN"], prep["colN"]] + eF[prep["rowF"], prep["colF"]]
    return out.astype(np.float32)


def kernel(qi, edge_dist, edge_index, q_ref, N, atom_mol_batch):
    prep = _prep(qi, edge_dist, edge_index, q_ref, N, atom_mol_batch)
    runner = _get_runner(prep["KsN"], prep["KsF"])
    maps = _shard_maps(prep)
    # Deterministic computation: rerun until two consecutive results agree
    # bit-exactly (the axon tunnel occasionally corrupts a dispatch).
    prev = None
    for _ in range(5):
        args = runner.put_inputs(maps)
        res = runner.results(runner.run(args))
        out = _unshard(prep, res)
        if prev is not None and np.array_equal(out, prev):
            return out
        prev = out
    return out


# revision 6
# speedup vs baseline: 1.1295x; 1.0498x over previous
"""CoulombLayer Trainium2 kernel v2 (8 NeuronCores, SPMD via bass).

Strategy vs v1 (baseline):
  * Edges are split on host into NEAR (d < 5, inside the smooth-cutoff
    transition of f(2d) with cutoff=10) and FAR (d >= 5, where the PhysNet
    cutoff f is exactly 0 so chi(d) = 1/d exactly).  69% of edges are far
    and need only a reciprocal and a multiply - no sqrt / smoothstep math.
  * Each stream gets its own vertex-cut CSR layout (degree-sorted atoms,
    dealt round-robin to cores, chunked with shared slot width K_c), its own
    accumulator grid and its own output; the host adds the two grids while
    unsharding (index-driven gather it already does anyway).
  * Reciprocals run as: ACT seed y = exp(-ln x) (natural_log_exp table set)
    + ONE fused Newton step on DVE via the custom-DVE op
    RECIPROCAL_APPROX_NR: ir = (2 - x*y)*y  (1 instruction instead of 2).
  * The near smoothstep g = f(z)*dm, f = z^3*((sqrt6 z - c)^2 + 0.625), is
    ONE authored custom-DVE instruction (F_SMOOTH_MUL, 8 ALU stages), with
    z = relu(1 - d/5) from ACT and dm = d - s from GPSIMD.
  * Engine balance per core (est): DVE ~52us, ACT ~50us, GPSIMD ~49us,
    DMA ~48us (fp32 memory floor).  Everything fp32: the 2e-2 max-rel-err
    gate is dominated by atoms with catastrophic term cancellation, which
    demands ~1e-7 per-term accuracy - no fp16/bf16 and no approximations
    beyond few-ulp.
  * ACT table sets: phase 1 (near s/z/p math) uses sqrt_and_others; phase 2
    (all seeds) uses natural_log_exp_and_others -> exactly 2 table loads per
    execution.
"""

import json as _json
import numpy as np

N_CORES = 8
N_ATOMS = 500_000
N_MOL = 5_000
N_EDGES = 16_000_000
CUTOFF = 10.0
P = 125                 # SBUF partitions used (125 * 500 = 62500 atoms/core)
APP = 500               # atoms per partition
CAT = 25                # atoms per compute tile (per partition)
GCAP = 2050             # max elements per grouped elementwise op

SQ6 = 6.0 ** 0.5
C15 = 15.0 / (2.0 * SQ6)

_RUNNER_CACHE = {}


# ---------------------------------------------------------------------------
# authored custom DVE ops (registered into concourse.dve_ops at first use):
#   F_SMOOTH_MUL_ANT: g = f(z)*dm, f = ((sqrt6*z - c)^2 + 0.625)*z^3 (the
#     PhysNet quintic smoothstep in z = relu(1 - d/5) form). 8 ALU stages,
#     one DVE pass instead of TT+STT+TT plus two ACT squares.
#   RECIP_NR2_ANT: two fused Newton steps y <- y*(2 - x*y) (6 stages). From
#     the ~1.2e-3 ACT exp(-ln x) seed this converges to the fp32 rounding
#     floor (~1.4e-7) in ONE DVE pass - the single-NR version left ~1e-6
#     per-term error, which the max-rel-err gate amplifies via cancellation.
# ---------------------------------------------------------------------------
_CUSTOM_OPS = None


def _get_custom_ops():
    global _CUSTOM_OPS
    if _CUSTOM_OPS is not None:
        return _CUSTOM_OPS
    import concourse.dve_ops as dve_ops
    from concourse.dve_spec import Spec, Src0, Src1, C0, C1, C2, lower, sq
    from concourse.dve_uop import DveOpSpec

    def _register(name, spec):
        if name in dve_ops._SUB_OPCODE_FOR_NAME:
            return next(op for op in dve_ops.OPS if op.name == name)
        row = max(dve_ops._SUB_OPCODE_FOR_NAME.values()) + 1
        assert row < 0x20
        shas = {}
        for ver in ("v3", "v4"):
            uops = lower(spec, ver=ver)
            shas[ver] = DveOpSpec(name=name, opcode=row, uops=uops,
                                  rd1_en=True).sha(ver)
        op = dve_ops.DveOp(name=name, spec=spec, subdim=False, uops_sha=shas)
        dve_ops._SUB_OPCODE_FOR_NAME[name] = row
        dve_ops.OPS.append(op)
        dve_ops.CUSTOM_DVE_SPECS[name] = spec
        return op

    def _fmul_ref(in0, in1, s0, s1, imm2):
        z = in0.astype(np.float32)
        f = ((z * s0 - s1) ** 2 + imm2) * z * z * z
        return (f * in1).astype(np.float32)

    fmul = _register("F_SMOOTH_MUL_ANT", Spec(
        body=(sq(Src0 * C0 - C1) + C2) * sq(Src0) * Src0 * Src1,
        reference=_fmul_ref,
    ))

    def _nr2_ref(in0, in1, s0, s1, imm2):
        t = ((s0 - in0 * in1) * in1).astype(np.float32)
        return ((s0 - in0 * t) * t).astype(np.float32)

    _y1 = (C0 - Src0 * Src1) * Src1
    nr2 = _register("RECIP_NR2_ANT", Spec(
        body=(C0 - Src0 * _y1) * _y1,
        reference=_nr2_ref,
    ))
    _CUSTOM_OPS = (fmul, nr2)
    return _CUSTOM_OPS


# ---------------------------------------------------------------------------
# walrus compat: this build rejects >1 sync-wait per instruction.  Split
# overflow waits onto NoOps inserted immediately before, same engine/block.
# ---------------------------------------------------------------------------
def _fix_bir_json(bir_json):
    m = _json.loads(bir_json)
    for fn in m.get("functions", []):
        for blk in fn.get("blocks", []):
            out = []
            for inst in blk.get("instructions", []):
                si = inst.get("sync_info")
                waits = (si or {}).get("on_wait", [])
                if len(waits) > 1:
                    for k, w in enumerate(waits[:-1]):
                        out.append({
                            "debug": inst.get("debug", 0),
                            "engine": inst["engine"],
                            "ins": [],
                            "name": f"{inst['name']}-sw{k}",
                            "opcode": "NoOp",
                            "outs": [],
                            "sync_info": {"on_update": [], "on_wait": [w]},
                        })
                    si["on_wait"] = [waits[-1]]
                out.append(inst)
            blk["instructions"] = out
    return _json.dumps(m).encode()


_PATCHED = False


def _install_compat():
    global _PATCHED
    if _PATCHED:
        return
    _PATCHED = True
    import concourse.bass_utils as bu
    import concourse.bass2jax as b2j
    orig = bu.compile_bir_kernel

    def patched(bir_json, tmpdir, neff_name="file.neff"):
        return orig(_fix_bir_json(bir_json), tmpdir, neff_name)

    bu.compile_bir_kernel = patched
    b2j.compile_bir_kernel = patched


def _groups(Ks):
    """Greedy-pack consecutive chunks into groups of total width <= GCAP."""
    out = []
    cur = [0]
    w = CAT * Ks[0]
    for c in range(1, len(Ks)):
        f = CAT * Ks[c]
        if w + f <= GCAP:
            cur.append(c)
            w += f
        else:
            out.append(cur)
            cur = [c]
            w = f
    out.append(cur)
    return out


# ---------------------------------------------------------------------------
# device program
# ---------------------------------------------------------------------------
def _build_nc(KsN, KsF, reps=1, body_mult=1,
              p_on_act=False, dm_on_gps=True, yn_on_gps=True, yf_on_gps=True,
              seed_dve=False, dma_only=False, tiny_dma=False,
              io_bufs=2, tmp_bufs=2):
    """Two-stream device program for one core.

    KsN/KsF: per-chunk slot widths for the near / far CSR layouts."""
    import concourse.bass as bass
    import concourse.mybir as mybir
    import concourse.tile as tile

    fmul, nr2 = _get_custom_ops()

    WN = sum(CAT * k for k in KsN)
    WF = sum(CAT * k for k in KsF)
    offsN = np.cumsum([0] + [CAT * k for k in KsN])
    offsF = np.cumsum([0] + [CAT * k for k in KsF])
    nc = bass.Bass()

    # const-AP pool entries for non-Copy activation biases (only 0.0/1.0 are
    # pre-registered)
    need_consts = [-0.25, 0.5] if p_on_act else []
    for v in need_consts:
        if (mybir.dt.float32, v) not in nc.const_aps.aps:
            ct = nc.alloc_sbuf_tensor(f"const-float32-{v}", [128, 1],
                                      mybir.dt.float32)
            nc.gpsimd.memset(ct.ap(), v)
            nc.const_aps.aps[(mybir.dt.float32, v)] = ct.ap()
    if need_consts:
        nc.all_engine_barrier()

    dqN_in = nc.declare_dram_parameter("dqN", [P, 2 * WN], mybir.dt.float32,
                                       isOutput=False)
    dqF_in = nc.declare_dram_parameter("dqF", [P, 2 * WF], mybir.dt.float32,
                                       isOutput=False)
    qicN_in = nc.declare_dram_parameter("qicN", [P, APP], mybir.dt.float32,
                                        isOutput=False)
    qicF_in = nc.declare_dram_parameter("qicF", [P, APP], mybir.dt.float32,
                                        isOutput=False)
    eN_out = nc.declare_dram_parameter("EN", [P, APP], mybir.dt.float32,
                                       isOutput=True)
    eF_out = nc.declare_dram_parameter("EF", [P, APP], mybir.dt.float32,
                                       isOutput=True)

    AL = mybir.AluOpType
    AF = mybir.ActivationFunctionType

    groupsN = _groups(KsN)
    groupsF = _groups(KsF)

    with tile.TileContext(nc, num_cores=N_CORES) as tc:
        with tc.tile_pool(name="io", bufs=io_bufs) as io, \
             tc.tile_pool(name="tmp", bufs=tmp_bufs) as tp, \
             tc.tile_pool(name="keep", bufs=len(groupsN)) as kp, \
             tc.tile_pool(name="accp", bufs=1) as ap_pool, \
             tc.tile_pool(name="qicp", bufs=1) as qp_pool:
            qicN = qp_pool.tile([P, APP], mybir.dt.float32, tag="qicN")
            qicF = qp_pool.tile([P, APP], mybir.dt.float32, tag="qicF")
            nc.scalar.dma_start(qicN[:], qicN_in[:])
            nc.scalar.dma_start(qicF[:], qicF_in[:])
            accN = ap_pool.tile([P, APP], mybir.dt.float32, tag="accN")
            accF = ap_pool.tile([P, APP], mybir.dt.float32, tag="accF")

            def body():
                # ---- phase 1: near geometry (sqrt_and_others table set) ----
                ph1 = []  # per near group: (p, num, Qt, width, chunks)
                fence_srcs = []
                for grp in groupsN:
                    F = sum(CAT * KsN[c] for c in grp)
                    o = 2 * int(offsN[grp[0]])
                    Dt = io.tile([P, F], mybir.dt.float32, tag="D")
                    Qt = kp.tile([P, F], mybir.dt.float32, tag="QN")
                    if tiny_dma:
                        nc.sync.dma_start(Dt[:, :64], dqN_in[:, o:o + 64])
                        nc.gpsimd.dma_start(Qt[:, :64],
                                            dqN_in[:, o + F:o + F + 64])
                    else:
                        nc.sync.dma_start(Dt[:], dqN_in[:, o:o + F])
                        nc.gpsimd.dma_start(Qt[:], dqN_in[:, o + F:o + 2 * F])
                    if dma_only:
                        ph1.append((None, None, Qt, F, grp))
                        continue
                    D = Dt[:]
                    t = tp.tile([P, F], mybir.dt.float32, tag="t")
                    z = tp.tile([P, F], mybir.dt.float32, tag="z")
                    dm = tp.tile([P, F], mybir.dt.float32, tag="dm")
                    p = kp.tile([P, F], mybir.dt.float32, tag="p")
                    num = kp.tile([P, F], mybir.dt.float32, tag="num")

                    nc.scalar.activation(t[:], D, AF.Square)
                    if p_on_act:
                        # d*s = sqrt((d^2+0.5)^2 - 0.25)
                        nc.scalar.activation(p[:], t[:], AF.Square, bias=0.5)
                        nc.scalar.activation(p[:], p[:], AF.Sqrt, bias=-0.25)
                    # near stream has d < 5 strictly (padding d=1), so
                    # z = 1 - d/5 > 0 always - no relu needed
                    nc.vector.tensor_scalar(z[:], D, -2.0 / CUTOFF, 1.0,
                                            op0=AL.mult, op1=AL.add)
                    # s overwrites t in place (ACT in-place is safe)
                    nc.scalar.activation(t[:], t[:], AF.Sqrt, bias=1.0)
                    s = t
                    if not p_on_act:
                        nc.gpsimd.tensor_tensor(p[:], D, s[:], op=AL.mult)
                    if dm_on_gps:
                        nc.gpsimd.tensor_tensor(dm[:], D, s[:], op=AL.subtract)
                    else:
                        nc.vector.tensor_tensor(dm[:], D, s[:], op=AL.subtract)
                    # g1 = f(z) * dm in one fused DVE pass (in place over z)
                    nc.vector._custom_dve(fmul, out=z[:], in0=z[:], in1=dm[:],
                                          s0=SQ6, s1=C15, imm2=0.625)
                    nc.vector.tensor_tensor(num[:], z[:], s[:], op=AL.add)
                    ph1.append((p, num, Qt, F, grp))
                    fence_srcs.append(s)

                # ---- phase 2: seeds + Newton + scatter (natural_log_exp) ----
                # Fence: a [P,1] exact-0.0 tile whose producer chain reads one
                # column of every phase-1 Sqrt output.  Used as the bias AP of
                # every Ln, it forces the scheduler to keep ALL sqrt-set ACT
                # ops before ALL natural_log-set ops (the engine runs in
                # order, so interleaving would re-load tables ~2.7us a pop).
                fence = None
                if not dma_only:
                    for s_t in fence_srcs:
                        fence_new = tp.tile([P, 1], mybir.dt.float32,
                                            tag="fence")
                        nc.scalar.activation(
                            fence_new[:], s_t[:, 0:1],
                            AF.Identity, scale=0.0,
                            bias=fence[:] if fence is not None else 0.0)
                        fence = fence_new
                for p, num, Qt, F, grp in ph1:
                    if dma_only:
                        loc = 0
                        for c in grp:
                            K = KsN[c]
                            Fc = CAT * K
                            nc.vector.tensor_reduce(
                                accN[:, c * CAT:(c + 1) * CAT],
                                Qt[:, loc:loc + Fc].rearrange(
                                    "p (a k) -> p a k", k=K),
                                axis=mybir.AxisListType.X,
                                op=AL.add,
                            )
                            loc += Fc
                        continue
                    y = tp.tile([P, F], mybir.dt.float32, tag="y")
                    Yt = tp.tile([P, F], mybir.dt.float32, tag="Y")
                    if seed_dve:
                        nc.vector.reciprocal_approx_fast(y[:], p[:])
                    else:
                        nc.scalar.activation(y[:], p[:], AF.Ln,
                                             bias=fence[:] if fence is not None
                                             else 0.0)
                        nc.scalar.activation(y[:], y[:], AF.Exp, scale=-1.0)
                    # ir = double-Newton 1/p, in place over y
                    nc.vector._custom_dve(nr2, out=y[:],
                                          in0=p[:], in1=y[:], s0=2.0)
                    if yn_on_gps:
                        nc.gpsimd.tensor_tensor(Yt[:], y[:], Qt[:], op=AL.mult)
                    else:
                        nc.vector.tensor_tensor(Yt[:], y[:], Qt[:], op=AL.mult)
                    nc.vector.tensor_tensor(Yt[:], num[:], Yt[:], op=AL.mult)
                    loc = 0
                    for c in grp:
                        K = KsN[c]
                        Fc = CAT * K
                        nc.vector.tensor_reduce(
                            accN[:, c * CAT:(c + 1) * CAT],
                            Yt[:, loc:loc + Fc].rearrange(
                                "p (a k) -> p a k", k=K),
                            axis=mybir.AxisListType.X,
                            op=AL.add,
                        )
                        loc += Fc

                for grp in groupsF:
                    F = sum(CAT * KsF[c] for c in grp)
                    o = 2 * int(offsF[grp[0]])
                    Dt = io.tile([P, F], mybir.dt.float32, tag="D")
                    Qt = io.tile([P, F], mybir.dt.float32, tag="Q")
                    if tiny_dma:
                        nc.sync.dma_start(Dt[:, :64], dqF_in[:, o:o + 64])
                        nc.gpsimd.dma_start(Qt[:, :64],
                                            dqF_in[:, o + F:o + F + 64])
                    else:
                        nc.sync.dma_start(Dt[:], dqF_in[:, o:o + F])
                        nc.gpsimd.dma_start(Qt[:], dqF_in[:, o + F:o + 2 * F])
                    D = Dt[:]
                    if dma_only:
                        loc = 0
                        for c in grp:
                            K = KsF[c]
                            Fc = CAT * K
                            nc.vector.tensor_reduce(
                                accF[:, c * CAT:(c + 1) * CAT],
                                Qt[:, loc:loc + Fc].rearrange(
                                    "p (a k) -> p a k", k=K),
                                axis=mybir.AxisListType.X,
                                op=AL.add,
                            )
                            loc += Fc
                        continue
                    y = tp.tile([P, F], mybir.dt.float32, tag="y")
                    Yt = tp.tile([P, F], mybir.dt.float32, tag="Y")
                    if seed_dve:
                        nc.vector.reciprocal_approx_fast(y[:], D)
                    else:
                        nc.scalar.activation(y[:], D, AF.Ln,
                                             bias=fence[:] if fence is not None
                                             else 0.0)
                        nc.scalar.activation(y[:], y[:], AF.Exp, scale=-1.0)
                    nc.vector._custom_dve(nr2, out=y[:],
                                          in0=D, in1=y[:], s0=2.0)
                    if yf_on_gps:
                        nc.gpsimd.tensor_tensor(Yt[:], y[:], Qt[:], op=AL.mult)
                    else:
                        nc.vector.tensor_tensor(Yt[:], y[:], Qt[:], op=AL.mult)
                    loc = 0
                    for c in grp:
                        K = KsF[c]
                        Fc = CAT * K
                        nc.vector.tensor_reduce(
                            accF[:, c * CAT:(c + 1) * CAT],
                            Yt[:, loc:loc + Fc].rearrange(
                                "p (a k) -> p a k", k=K),
                            axis=mybir.AxisListType.X,
                            op=AL.add,
                        )
                        loc += Fc

                # E = acc * qic (qic pre-scaled by +0.5*qi_c on host)
                nc.vector.tensor_tensor(accN[:], accN[:], qicN[:], op=AL.mult)
                nc.vector.tensor_tensor(accF[:], accF[:], qicF[:], op=AL.mult)
                nc.scalar.dma_start(eN_out[:], accN[:])
                nc.scalar.dma_start(eF_out[:], accF[:])

            if reps == 1:
                body()
            else:
                with tc.For_i(0, reps):
                    for _ in range(body_mult):
                        body()
    # populate .instr bytes for InstISA subclasses (custom DVE ops); without
    # this walrus fails with "ISA wrong length"
    mybir.codegen_inst_isa_subclasses(nc)
    return nc


class _Runner:
    """Compile once; keep a reusable jitted SPMD callable."""

    def __init__(self, nc):
        import jax
        from jax.sharding import Mesh, PartitionSpec, NamedSharding
        from jax.experimental.shard_map import shard_map
        import concourse.mybir as mybir
        import concourse.bass2jax as b2j
        b2j.install_neuronx_cc_hook()
        self.jax = jax
        in_names, out_names, out_avals, zero_outs = [], [], [], []
        pname = nc.partition_id_tensor.name if nc.partition_id_tensor else None
        for alloc in nc.m.functions[0].allocations:
            if not isinstance(alloc, mybir.MemoryLocationSet):
                continue
            name = alloc.memorylocations[0].name
            if alloc.kind == "ExternalInput":
                if name != pname:
                    in_names.append(name)
            elif alloc.kind == "ExternalOutput":
                shape = tuple(alloc.tensor_shape)
                dtype = mybir.dt.np(alloc.dtype)
                out_names.append(name)
                out_avals.append(jax.core.ShapedArray(shape, dtype))
                zero_outs.append(np.zeros(shape, dtype))
        self.in_names, self.out_names = in_names, out_names
        self.out_avals, self.zero_outs = out_avals, zero_outs
        all_in = list(in_names) + list(out_names) + ([pname] if pname else [])

        def _body(*args):
            operands = list(args)
            if pname is not None:
                operands.append(b2j.partition_id_tensor())
            return tuple(b2j._bass_exec_p.bind(
                *operands,
                out_avals=tuple(out_avals),
                in_names=tuple(all_in),
                out_names=tuple(out_names),
                lowering_input_output_aliases=(),
                sim_require_finite=True,
                sim_require_nnan=True,
                nc=nc,
            ))

        devices = jax.devices()[:N_CORES]
        mesh = Mesh(np.asarray(devices), ("core",))
        n_in = len(in_names) + len(zero_outs)
        self.fn = jax.jit(
            shard_map(_body, mesh=mesh,
                      in_specs=(PartitionSpec("core"),) * n_in,
                      out_specs=(PartitionSpec("core"),) * len(out_names),
                      check_rep=False),
            keep_unused=True,
        )
        self.sharding = NamedSharding(mesh, PartitionSpec("core"))

    def put_inputs(self, in_maps, device_resident=False):
        args = []
        for name in self.in_names:
            cat = np.concatenate([np.asarray(m[name]) for m in in_maps], axis=0)
            args.append(cat)
        for z in self.zero_outs:
            args.append(np.zeros((N_CORES * z.shape[0], *z.shape[1:]), z.dtype))
        if device_resident:
            try:
                jax = self.jax
                devices = list(self.sharding.mesh.devices.reshape(-1))
                put = []
                for a in args:
                    per = a.shape[0] // N_CORES
                    shards = [
                        jax.device_put(a[c * per:(c + 1) * per], devices[c])
                        for c in range(N_CORES)
                    ]
                    put.append(jax.make_array_from_single_device_arrays(
                        a.shape, self.sharding, shards))
                jax.block_until_ready(put)
                args = put
            except Exception:
                pass
        return args

    def run(self, args):
        outs = self.fn(*args)
        self.jax.block_until_ready(outs)
        return outs

    def results(self, outs):
        res = []
        for c in range(N_CORES):
            res.append({
                name: np.asarray(outs[i]).reshape(N_CORES, *self.out_avals[i].shape)[c]
                for i, name in enumerate(self.out_names)
            })
        return res


def _get_runner(KsN, KsF, reps=1, body_mult=1, **bk):
    key = (tuple(KsN), tuple(KsF), reps, body_mult, tuple(sorted(bk.items())))
    if key not in _RUNNER_CACHE:
        _install_compat()
        _RUNNER_CACHE[key] = _Runner(
            _build_nc(tuple(KsN), tuple(KsF), reps, body_mult, **bk))
    return _RUNNER_CACHE[key]


# ---------------------------------------------------------------------------
# host-side shard construction: one degree-bucketed CSR layout per stream
# ---------------------------------------------------------------------------
def _stream_layout(ii_sub, d_sub, qj_sub):
    """Build the [N_CORES*P, 2W] interleaved (d, qj) grid + per-atom output
    position maps for one edge subset (edges targeting atom ii_sub)."""
    counts = np.bincount(ii_sub, minlength=N_ATOMS)
    a_order = np.argsort(-counts, kind="stable")
    degs = counts[a_order]
    n_chunks = APP // CAT
    cg = N_CORES * P * CAT
    Ks = tuple(int(degs[c * cg:(c + 1) * cg].max()) for c in range(n_chunks))
    W = sum(CAT * k for k in Ks)
    offs_c = np.cumsum([0] + [CAT * k for k in Ks])

    groups = _groups(Ks)
    c0_of_c = np.empty(n_chunks, np.int64)
    fg_of_c = np.empty(n_chunks, np.int64)
    for g in groups:
        fg = sum(CAT * Ks[c] for c in g)
        for c in g:
            c0_of_c[c] = g[0]
            fg_of_c[c] = fg
    rank = np.arange(N_ATOMS, dtype=np.int64)
    core = rank % N_CORES
    r = rank // N_CORES
    c_of = r // (P * CAT)
    w = r % (P * CAT)
    p_of = w % P
    j_of = w // P
    row = core * P + p_of
    colE = c_of * CAT + j_of
    kc = np.asarray(Ks, np.int64)[c_of]
    dcol = offs_c[c_of] + offs_c[c0_of_c[c_of]]
    based = row * 2 * W + dcol + j_of * kc

    row_of = np.empty(N_ATOMS, np.int64)
    colE_of = np.empty(N_ATOMS, np.int64)
    based_of = np.empty(N_ATOMS, np.int64)
    fc_of = np.empty(N_ATOMS, np.int64)
    row_of[a_order] = row
    colE_of[a_order] = colE
    based_of[a_order] = based
    fc_of[a_order] = fg_of_c[c_of]

    e_order = np.argsort(ii_sub, kind="stable")
    i_s = ii_sub[e_order]
    csr = np.zeros(N_ATOMS, np.int64)
    np.cumsum(counts[:-1], out=csr[1:])
    slot = np.arange(len(ii_sub), dtype=np.int64) - csr[i_s]
    pos_d = based_of[i_s] + slot
    pos_q = pos_d + fc_of[i_s]

    dq = np.zeros((N_CORES * P, 2 * W), np.float32)
    for g in groups:
        o = 2 * int(offs_c[g[0]])
        fg = sum(CAT * Ks[c] for c in g)
        dq[:, o:o + fg] = 1.0            # d padding (avoid ln(0))
    dq = dq.reshape(-1)
    dq[pos_d] = d_sub[e_order]
    dq[pos_q] = qj_sub[e_order]
    return {
        "dq": dq.reshape(N_CORES * P, 2 * W),
        "Ks": Ks,
        "row_of": row_of,
        "colE_of": colE_of,
    }


def _prep(qi, edge_dist, edge_index, q_ref, N, atom_mol_batch):
    qi = np.asarray(qi, np.float32)
    edge_dist = np.asarray(edge_dist, np.float32)
    ii = np.asarray(edge_index[0], np.int64)
    jj = np.asarray(edge_index[1], np.int64)
    # charge-neutrality correction (index-driven segment sum over atoms)
    q_mol = np.bincount(np.asarray(atom_mol_batch, np.int64), weights=qi,
                        minlength=N_MOL).astype(np.float32)
    corr = (q_mol - np.asarray(q_ref, np.float32)) / np.asarray(N, np.float32)
    qi_c = qi - corr[np.asarray(atom_mol_batch, np.int64)]
    qj_c = qi_c[jj]

    near = edge_dist < (CUTOFF / 2.0)
    farm = ~near
    LN = _stream_layout(ii[near], edge_dist[near], qj_c[near])
    LF = _stream_layout(ii[farm], edge_dist[farm], qj_c[farm])

    # qic grids pre-scaled by +0.5*qi_c (0.5 = double-counting factor; the
    # NR reciprocal is positive, unlike v1's negated form)
    qic = qi_c * np.float32(0.5)
    qicN = np.zeros((N_CORES * P, APP), np.float32)
    qicF = np.zeros((N_CORES * P, APP), np.float32)
    qicN[LN["row_of"], LN["colE_of"]] = qic
    qicF[LF["row_of"], LF["colE_of"]] = qic
    return {
        "dqN": LN["dq"], "dqF": LF["dq"],
        "KsN": LN["Ks"], "KsF": LF["Ks"],
        "qicN": qicN, "qicF": qicF,
        "rowN": LN["row_of"], "colN": LN["colE_of"],
        "rowF": LF["row_of"], "colF": LF["colE_of"],
    }


def _shard_maps(prep):
    in_maps = []
    for c in range(N_CORES):
        rs = slice(c * P, (c + 1) * P)
        in_maps.append({
            "dqN": prep["dqN"][rs],
            "dqF": prep["dqF"][rs],
            "qicN": prep["qicN"][rs],
            "qicF": prep["qicF"][rs],
        })
    return in_maps


def _unshard(prep, res):
    eN = np.concatenate([r["EN"].reshape(P, APP) for r in res], axis=0)
    eF = np.concatenate([r["EF"].reshape(P, APP) for r in res], axis=0)
    out = eN[prep["rowN"], prep["colN"]] + eF[prep["rowF"], prep["colF"]]
    return out.astype(np.float32)


def kernel(qi, edge_dist, edge_index, q_ref, N, atom_mol_batch):
    prep = _prep(qi, edge_dist, edge_index, q_ref, N, atom_mol_batch)
    runner = _get_runner(prep["KsN"], prep["KsF"])
    maps = _shard_maps(prep)
    # Deterministic computation: rerun until two consecutive results agree
    # bit-exactly (the axon tunnel occasionally corrupts a dispatch).
    prev = None
    for _ in range(5):
        args = runner.put_inputs(maps)
        res = runner.results(runner.run(args))
        out = _unshard(prep, res)
        if prev is not None and np.array_equal(out, prev):
            return out
        prev = out
    return out


# revision 9
# speedup vs baseline: 1.2441x; 1.1015x over previous
"""CoulombLayer Trainium2 kernel v2 (8 NeuronCores, SPMD via bass).

Strategy vs v1 (baseline):
  * Edges are split on host into NEAR (d < 5, inside the smooth-cutoff
    transition of f(2d) with cutoff=10) and FAR (d >= 5, where the PhysNet
    cutoff f is exactly 0 so chi(d) = 1/d exactly).  69% of edges are far
    and need only a reciprocal and a multiply - no sqrt / smoothstep math.
  * Each stream gets its own vertex-cut CSR layout (degree-sorted atoms,
    dealt round-robin to cores, chunked with shared slot width K_c), its own
    accumulator grid and its own output; the host adds the two grids while
    unsharding (index-driven gather it already does anyway).
  * Reciprocals run as: ACT seed y = exp(-ln x) (natural_log_exp table set)
    + ONE fused Newton step on DVE via the custom-DVE op
    RECIPROCAL_APPROX_NR: ir = (2 - x*y)*y  (1 instruction instead of 2).
  * The near smoothstep g = f(z)*dm, f = z^3*((sqrt6 z - c)^2 + 0.625), is
    ONE authored custom-DVE instruction (F_SMOOTH_MUL, 8 ALU stages), with
    z = relu(1 - d/5) from ACT and dm = d - s from GPSIMD.
  * Engine balance per core (est): DVE ~52us, ACT ~50us, GPSIMD ~49us,
    DMA ~48us (fp32 memory floor).  Everything fp32: the 2e-2 max-rel-err
    gate is dominated by atoms with catastrophic term cancellation, which
    demands ~1e-7 per-term accuracy - no fp16/bf16 and no approximations
    beyond few-ulp.
  * ACT table sets: phase 1 (near s/z/p math) uses sqrt_and_others; phase 2
    (all seeds) uses natural_log_exp_and_others -> exactly 2 table loads per
    execution.
"""

import json as _json
import numpy as np

N_CORES = 8
N_ATOMS = 500_000
N_MOL = 5_000
N_EDGES = 16_000_000
CUTOFF = 10.0
P = 125                 # SBUF partitions used (125 * 500 = 62500 atoms/core)
APP = 500               # atoms per partition
CAT = 25                # atoms per compute tile (per partition)
GCAP = 2050             # max elements per grouped elementwise op

SQ6 = 6.0 ** 0.5
C15 = 15.0 / (2.0 * SQ6)

_RUNNER_CACHE = {}


# ---------------------------------------------------------------------------
# authored custom DVE ops (registered into concourse.dve_ops at first use):
#   F_SMOOTH_MUL_ANT: g = f(z)*dm, f = ((sqrt6*z - c)^2 + 0.625)*z^3 (the
#     PhysNet quintic smoothstep in z = relu(1 - d/5) form). 8 ALU stages,
#     one DVE pass instead of TT+STT+TT plus two ACT squares.
#   RECIP_NR2_ANT: two fused Newton steps y <- y*(2 - x*y) (6 stages). From
#     the ~1.2e-3 ACT exp(-ln x) seed this converges to the fp32 rounding
#     floor (~1.4e-7) in ONE DVE pass - the single-NR version left ~1e-6
#     per-term error, which the max-rel-err gate amplifies via cancellation.
# ---------------------------------------------------------------------------
_CUSTOM_OPS = None


def _get_custom_ops():
    global _CUSTOM_OPS
    if _CUSTOM_OPS is not None:
        return _CUSTOM_OPS
    import concourse.dve_ops as dve_ops
    from concourse.dve_spec import Spec, Src0, Src1, C0, C1, C2, lower, sq
    from concourse.dve_uop import DveOpSpec

    def _register(name, spec):
        if name in dve_ops._SUB_OPCODE_FOR_NAME:
            return next(op for op in dve_ops.OPS if op.name == name)
        row = max(dve_ops._SUB_OPCODE_FOR_NAME.values()) + 1
        assert row < 0x20
        shas = {}
        for ver in ("v3", "v4"):
            uops = lower(spec, ver=ver)
            shas[ver] = DveOpSpec(name=name, opcode=row, uops=uops,
                                  rd1_en=True).sha(ver)
        op = dve_ops.DveOp(name=name, spec=spec, subdim=False, uops_sha=shas)
        dve_ops._SUB_OPCODE_FOR_NAME[name] = row
        dve_ops.OPS.append(op)
        dve_ops.CUSTOM_DVE_SPECS[name] = spec
        return op

    def _fmul_ref(in0, in1, s0, s1, imm2):
        z = in0.astype(np.float32)
        f = ((z * s0 - s1) ** 2 + imm2) * z * z * z
        return (f * in1).astype(np.float32)

    fmul = _register("F_SMOOTH_MUL_ANT", Spec(
        body=(sq(Src0 * C0 - C1) + C2) * sq(Src0) * Src0 * Src1,
        reference=_fmul_ref,
    ))

    def _nr2_ref(in0, in1, s0, s1, imm2):
        t = ((s0 - in0 * in1) * in1).astype(np.float32)
        return ((s0 - in0 * t) * t).astype(np.float32)

    _y1 = (C0 - Src0 * Src1) * Src1
    nr2 = _register("RECIP_NR2_ANT", Spec(
        body=(C0 - Src0 * _y1) * _y1,
        reference=_nr2_ref,
    ))
    _CUSTOM_OPS = (fmul, nr2)
    return _CUSTOM_OPS


# ---------------------------------------------------------------------------
# walrus compat: this build rejects >1 sync-wait per instruction.  Split
# overflow waits onto NoOps inserted immediately before, same engine/block.
# ---------------------------------------------------------------------------
def _fix_bir_json(bir_json):
    m = _json.loads(bir_json)
    for fn in m.get("functions", []):
        for blk in fn.get("blocks", []):
            out = []
            for inst in blk.get("instructions", []):
                si = inst.get("sync_info")
                waits = (si or {}).get("on_wait", [])
                if len(waits) > 1:
                    for k, w in enumerate(waits[:-1]):
                        out.append({
                            "debug": inst.get("debug", 0),
                            "engine": inst["engine"],
                            "ins": [],
                            "name": f"{inst['name']}-sw{k}",
                            "opcode": "NoOp",
                            "outs": [],
                            "sync_info": {"on_update": [], "on_wait": [w]},
                        })
                    si["on_wait"] = [waits[-1]]
                out.append(inst)
            blk["instructions"] = out
    return _json.dumps(m).encode()


_PATCHED = False


def _install_compat():
    global _PATCHED
    if _PATCHED:
        return
    _PATCHED = True
    import concourse.bass_utils as bu
    import concourse.bass2jax as b2j
    orig = bu.compile_bir_kernel

    def patched(bir_json, tmpdir, neff_name="file.neff"):
        return orig(_fix_bir_json(bir_json), tmpdir, neff_name)

    bu.compile_bir_kernel = patched
    b2j.compile_bir_kernel = patched


def _groups(Ks):
    """Greedy-pack consecutive chunks into groups of total width <= GCAP."""
    out = []
    cur = [0]
    w = CAT * Ks[0]
    for c in range(1, len(Ks)):
        f = CAT * Ks[c]
        if w + f <= GCAP:
            cur.append(c)
            w += f
        else:
            out.append(cur)
            cur = [c]
            w = f
    out.append(cur)
    return out


# ---------------------------------------------------------------------------
# device program
# ---------------------------------------------------------------------------
def _build_nc(KsN, KsF, reps=1, body_mult=1,
              p_on_act=False, dm_on_gps=True, yn_on_gps=True, yf_on_gps=True,
              seed_dve=False, dma_only=False, tiny_dma=False,
              io_bufs=3, tmp_bufs=2):
    """Two-stream device program for one core.

    KsN/KsF: per-chunk slot widths for the near / far CSR layouts."""
    import concourse.bass as bass
    import concourse.mybir as mybir
    import concourse.tile as tile

    fmul, nr2 = _get_custom_ops()

    WN = sum(CAT * k for k in KsN)
    WF = sum(CAT * k for k in KsF)
    offsN = np.cumsum([0] + [CAT * k for k in KsN])
    offsF = np.cumsum([0] + [CAT * k for k in KsF])
    nc = bass.Bass()

    # const-AP pool entries for non-Copy activation biases (only 0.0/1.0 are
    # pre-registered)
    need_consts = [-0.25, 0.5] if p_on_act else []
    for v in need_consts:
        if (mybir.dt.float32, v) not in nc.const_aps.aps:
            ct = nc.alloc_sbuf_tensor(f"const-float32-{v}", [128, 1],
                                      mybir.dt.float32)
            nc.gpsimd.memset(ct.ap(), v)
            nc.const_aps.aps[(mybir.dt.float32, v)] = ct.ap()
    if need_consts:
        nc.all_engine_barrier()

    dqN_in = nc.declare_dram_parameter("dqN", [P, 2 * WN], mybir.dt.float32,
                                       isOutput=False)
    dqF_in = nc.declare_dram_parameter("dqF", [P, 2 * WF], mybir.dt.float32,
                                       isOutput=False)
    qicN_in = nc.declare_dram_parameter("qicN", [P, APP], mybir.dt.float32,
                                        isOutput=False)
    qicF_in = nc.declare_dram_parameter("qicF", [P, APP], mybir.dt.float32,
                                        isOutput=False)
    eN_out = nc.declare_dram_parameter("EN", [P, APP], mybir.dt.float32,
                                       isOutput=True)
    eF_out = nc.declare_dram_parameter("EF", [P, APP], mybir.dt.float32,
                                       isOutput=True)

    AL = mybir.AluOpType
    AF = mybir.ActivationFunctionType

    groupsN = _groups(KsN)
    groupsF = _groups(KsF)

    with tile.TileContext(nc, num_cores=N_CORES) as tc:
        with tc.tile_pool(name="io", bufs=io_bufs) as io, \
             tc.tile_pool(name="tmp", bufs=tmp_bufs) as tp, \
             tc.tile_pool(name="keep", bufs=len(groupsN)) as kp, \
             tc.tile_pool(name="accp", bufs=2) as ap_pool, \
             tc.tile_pool(name="qicp", bufs=1) as qp_pool:
            qicN = qp_pool.tile([P, APP], mybir.dt.float32, tag="qicN")
            qicF = qp_pool.tile([P, APP], mybir.dt.float32, tag="qicF")
            nc.scalar.dma_start(qicN[:], qicN_in[:])
            nc.scalar.dma_start(qicF[:], qicF_in[:])
            def body():
                # fresh accumulator generation each iteration (bufs=2): the
                # next iteration's reduces don't wait for this iteration's
                # final scale + output DMA
                accN = ap_pool.tile([P, APP], mybir.dt.float32, tag="accN")
                accF = ap_pool.tile([P, APP], mybir.dt.float32, tag="accF")
                # ---- phase 1: near geometry (sqrt_and_others table set) ----
                ph1 = []  # per near group: (p, num, Qt, width, chunks)
                fence_srcs = []
                for grp in groupsN:
                    F = sum(CAT * KsN[c] for c in grp)
                    o = 2 * int(offsN[grp[0]])
                    Dt = io.tile([P, F], mybir.dt.float32, tag="D")
                    Qt = kp.tile([P, F], mybir.dt.float32, tag="QN")
                    if tiny_dma:
                        nc.sync.dma_start(Dt[:, :64], dqN_in[:, o:o + 64])
                        nc.gpsimd.dma_start(Qt[:, :64],
                                            dqN_in[:, o + F:o + F + 64])
                    else:
                        nc.sync.dma_start(Dt[:], dqN_in[:, o:o + F])
                        nc.gpsimd.dma_start(Qt[:], dqN_in[:, o + F:o + 2 * F])
                    if dma_only:
                        ph1.append((None, None, Qt, F, grp))
                        continue
                    D = Dt[:]
                    t = tp.tile([P, F], mybir.dt.float32, tag="t")
                    z = tp.tile([P, F], mybir.dt.float32, tag="z")
                    dm = tp.tile([P, F], mybir.dt.float32, tag="dm")
                    p = kp.tile([P, F], mybir.dt.float32, tag="p")
                    num = kp.tile([P, F], mybir.dt.float32, tag="num")

                    nc.scalar.activation(t[:], D, AF.Square)
                    if p_on_act:
                        # d*s = sqrt((d^2+0.5)^2 - 0.25)
                        nc.scalar.activation(p[:], t[:], AF.Square, bias=0.5)
                        nc.scalar.activation(p[:], p[:], AF.Sqrt, bias=-0.25)
                    # near stream has d < 5 strictly (padding d=1), so
                    # z = 1 - d/5 > 0 always - no relu needed
                    nc.vector.tensor_scalar(z[:], D, -2.0 / CUTOFF, 1.0,
                                            op0=AL.mult, op1=AL.add)
                    # s overwrites t in place (ACT in-place is safe)
                    nc.scalar.activation(t[:], t[:], AF.Sqrt, bias=1.0)
                    s = t
                    if not p_on_act:
                        nc.gpsimd.tensor_tensor(p[:], D, s[:], op=AL.mult)
                    if dm_on_gps:
                        nc.gpsimd.tensor_tensor(dm[:], D, s[:], op=AL.subtract)
                    else:
                        nc.vector.tensor_tensor(dm[:], D, s[:], op=AL.subtract)
                    # g1 = f(z) * dm in one fused DVE pass (in place over z)
                    nc.vector._custom_dve(fmul, out=z[:], in0=z[:], in1=dm[:],
                                          s0=SQ6, s1=C15, imm2=0.625)
                    nc.vector.tensor_tensor(num[:], z[:], s[:], op=AL.add)
                    ph1.append((p, num, Qt, F, grp))
                    fence_srcs.append(s)

                # ---- phase 2: seeds + Newton + scatter (natural_log_exp) ----
                # Fence: a [P,1] exact-0.0 tile whose producer chain reads one
                # column of every phase-1 Sqrt output.  Used as the bias AP of
                # every Ln, it forces the scheduler to keep ALL sqrt-set ACT
                # ops before ALL natural_log-set ops (the engine runs in
                # order, so interleaving would re-load tables ~2.7us a pop).
                fence = None
                if not dma_only:
                    for s_t in fence_srcs:
                        fence_new = tp.tile([P, 1], mybir.dt.float32,
                                            tag="fence")
                        nc.scalar.activation(
                            fence_new[:], s_t[:, 0:1],
                            AF.Identity, scale=0.0,
                            bias=fence[:] if fence is not None else 0.0)
                        fence = fence_new
                for p, num, Qt, F, grp in ph1:
                    if dma_only:
                        loc = 0
                        for c in grp:
                            K = KsN[c]
                            Fc = CAT * K
                            nc.vector.tensor_reduce(
                                accN[:, c * CAT:(c + 1) * CAT],
                                Qt[:, loc:loc + Fc].rearrange(
                                    "p (a k) -> p a k", k=K),
                                axis=mybir.AxisListType.X,
                                op=AL.add,
                            )
                            loc += Fc
                        continue
                    y = tp.tile([P, F], mybir.dt.float32, tag="y")
                    Yt = tp.tile([P, F], mybir.dt.float32, tag="Y")
                    if seed_dve:
                        nc.vector.reciprocal_approx_fast(y[:], p[:])
                    else:
                        nc.scalar.activation(y[:], p[:], AF.Ln,
                                             bias=fence[:] if fence is not None
                                             else 0.0)
                        nc.scalar.activation(y[:], y[:], AF.Exp, scale=-1.0)
                    # ir = double-Newton 1/p, in place over y
                    nc.vector._custom_dve(nr2, out=y[:],
                                          in0=p[:], in1=y[:], s0=2.0)
                    if yn_on_gps:
                        nc.gpsimd.tensor_tensor(Yt[:], y[:], Qt[:], op=AL.mult)
                    else:
                        nc.vector.tensor_tensor(Yt[:], y[:], Qt[:], op=AL.mult)
                    nc.vector.tensor_tensor(Yt[:], num[:], Yt[:], op=AL.mult)
                    loc = 0
                    for c in grp:
                        K = KsN[c]
                        Fc = CAT * K
                        nc.vector.tensor_reduce(
                            accN[:, c * CAT:(c + 1) * CAT],
                            Yt[:, loc:loc + Fc].rearrange(
                                "p (a k) -> p a k", k=K),
                            axis=mybir.AxisListType.X,
                            op=AL.add,
                        )
                        loc += Fc

                for grp in groupsF:
                    F = sum(CAT * KsF[c] for c in grp)
                    o = 2 * int(offsF[grp[0]])
                    Dt = io.tile([P, F], mybir.dt.float32, tag="D")
                    Qt = io.tile([P, F], mybir.dt.float32, tag="Q")
                    if tiny_dma:
                        nc.sync.dma_start(Dt[:, :64], dqF_in[:, o:o + 64])
                        nc.gpsimd.dma_start(Qt[:, :64],
                                            dqF_in[:, o + F:o + F + 64])
                    else:
                        nc.sync.dma_start(Dt[:], dqF_in[:, o:o + F])
                        nc.gpsimd.dma_start(Qt[:], dqF_in[:, o + F:o + 2 * F])
                    D = Dt[:]
                    if dma_only:
                        loc = 0
                        for c in grp:
                            K = KsF[c]
                            Fc = CAT * K
                            nc.vector.tensor_reduce(
                                accF[:, c * CAT:(c + 1) * CAT],
                                Qt[:, loc:loc + Fc].rearrange(
                                    "p (a k) -> p a k", k=K),
                                axis=mybir.AxisListType.X,
                                op=AL.add,
                            )
                            loc += Fc
                        continue
                    y = tp.tile([P, F], mybir.dt.float32, tag="y")
                    Yt = tp.tile([P, F], mybir.dt.float32, tag="Y")
                    if seed_dve:
                        nc.vector.reciprocal_approx_fast(y[:], D)
                    else:
                        nc.scalar.activation(y[:], D, AF.Ln,
                                             bias=fence[:] if fence is not None
                                             else 0.0)
                        nc.scalar.activation(y[:], y[:], AF.Exp, scale=-1.0)
                    nc.vector._custom_dve(nr2, out=y[:],
                                          in0=D, in1=y[:], s0=2.0)
                    if yf_on_gps:
                        nc.gpsimd.tensor_tensor(Yt[:], y[:], Qt[:], op=AL.mult)
                    else:
                        nc.vector.tensor_tensor(Yt[:], y[:], Qt[:], op=AL.mult)
                    loc = 0
                    for c in grp:
                        K = KsF[c]
                        Fc = CAT * K
                        nc.vector.tensor_reduce(
                            accF[:, c * CAT:(c + 1) * CAT],
                            Yt[:, loc:loc + Fc].rearrange(
                                "p (a k) -> p a k", k=K),
                            axis=mybir.AxisListType.X,
                            op=AL.add,
                        )
                        loc += Fc

                # E = acc * qic (qic pre-scaled by +0.5*qi_c on host)
                nc.vector.tensor_tensor(accN[:], accN[:], qicN[:], op=AL.mult)
                nc.vector.tensor_tensor(accF[:], accF[:], qicF[:], op=AL.mult)
                nc.scalar.dma_start(eN_out[:], accN[:])
                nc.scalar.dma_start(eF_out[:], accF[:])

            if reps == 1:
                body()
            else:
                with tc.For_i(0, reps):
                    for _ in range(body_mult):
                        body()
    # populate .instr bytes for InstISA subclasses (custom DVE ops); without
    # this walrus fails with "ISA wrong length"
    mybir.codegen_inst_isa_subclasses(nc)
    return nc


class _Runner:
    """Compile once; keep a reusable jitted SPMD callable."""

    def __init__(self, nc):
        import jax
        from jax.sharding import Mesh, PartitionSpec, NamedSharding
        from jax.experimental.shard_map import shard_map
        import concourse.mybir as mybir
        import concourse.bass2jax as b2j
        b2j.install_neuronx_cc_hook()
        self.jax = jax
        in_names, out_names, out_avals, zero_outs = [], [], [], []
        pname = nc.partition_id_tensor.name if nc.partition_id_tensor else None
        for alloc in nc.m.functions[0].allocations:
            if not isinstance(alloc, mybir.MemoryLocationSet):
                continue
            name = alloc.memorylocations[0].name
            if alloc.kind == "ExternalInput":
                if name != pname:
                    in_names.append(name)
            elif alloc.kind == "ExternalOutput":
                shape = tuple(alloc.tensor_shape)
                dtype = mybir.dt.np(alloc.dtype)
                out_names.append(name)
                out_avals.append(jax.core.ShapedArray(shape, dtype))
                zero_outs.append(np.zeros(shape, dtype))
        self.in_names, self.out_names = in_names, out_names
        self.out_avals, self.zero_outs = out_avals, zero_outs
        all_in = list(in_names) + list(out_names) + ([pname] if pname else [])

        def _body(*args):
            operands = list(args)
            if pname is not None:
                operands.append(b2j.partition_id_tensor())
            return tuple(b2j._bass_exec_p.bind(
                *operands,
                out_avals=tuple(out_avals),
                in_names=tuple(all_in),
                out_names=tuple(out_names),
                lowering_input_output_aliases=(),
                sim_require_finite=True,
                sim_require_nnan=True,
                nc=nc,
            ))

        devices = jax.devices()[:N_CORES]
        mesh = Mesh(np.asarray(devices), ("core",))
        n_in = len(in_names) + len(zero_outs)
        self.fn = jax.jit(
            shard_map(_body, mesh=mesh,
                      in_specs=(PartitionSpec("core"),) * n_in,
                      out_specs=(PartitionSpec("core"),) * len(out_names),
                      check_rep=False),
            keep_unused=True,
        )
        self.sharding = NamedSharding(mesh, PartitionSpec("core"))

    def put_inputs(self, in_maps, device_resident=False):
        args = []
        for name in self.in_names:
            cat = np.concatenate([np.asarray(m[name]) for m in in_maps], axis=0)
            args.append(cat)
        for z in self.zero_outs:
            args.append(np.zeros((N_CORES * z.shape[0], *z.shape[1:]), z.dtype))
        if device_resident:
            try:
                jax = self.jax
                devices = list(self.sharding.mesh.devices.reshape(-1))
                put = []
                for a in args:
                    per = a.shape[0] // N_CORES
                    shards = [
                        jax.device_put(a[c * per:(c + 1) * per], devices[c])
                        for c in range(N_CORES)
                    ]
                    put.append(jax.make_array_from_single_device_arrays(
                        a.shape, self.sharding, shards))
                jax.block_until_ready(put)
                args = put
            except Exception:
                pass
        return args

    def run(self, args):
        outs = self.fn(*args)
        self.jax.block_until_ready(outs)
        return outs

    def results(self, outs):
        res = []
        for c in range(N_CORES):
            res.append({
                name: np.asarray(outs[i]).reshape(N_CORES, *self.out_avals[i].shape)[c]
                for i, name in enumerate(self.out_names)
            })
        return res


def _get_runner(KsN, KsF, reps=1, body_mult=1, **bk):
    key = (tuple(KsN), tuple(KsF), reps, body_mult, tuple(sorted(bk.items())))
    if key not in _RUNNER_CACHE:
        _install_compat()
        _RUNNER_CACHE[key] = _Runner(
            _build_nc(tuple(KsN), tuple(KsF), reps, body_mult, **bk))
    return _RUNNER_CACHE[key]


# ---------------------------------------------------------------------------
# host-side shard construction: one degree-bucketed CSR layout per stream
# ---------------------------------------------------------------------------
def _stream_layout(ii_sub, d_sub, qj_sub):
    """Build the [N_CORES*P, 2W] interleaved (d, qj) grid + per-atom output
    position maps for one edge subset (edges targeting atom ii_sub)."""
    counts = np.bincount(ii_sub, minlength=N_ATOMS)
    a_order = np.argsort(-counts, kind="stable")
    degs = counts[a_order]
    n_chunks = APP // CAT
    cg = N_CORES * P * CAT
    Ks = tuple(int(degs[c * cg:(c + 1) * cg].max()) for c in range(n_chunks))
    W = sum(CAT * k for k in Ks)
    offs_c = np.cumsum([0] + [CAT * k for k in Ks])

    groups = _groups(Ks)
    c0_of_c = np.empty(n_chunks, np.int64)
    fg_of_c = np.empty(n_chunks, np.int64)
    for g in groups:
        fg = sum(CAT * Ks[c] for c in g)
        for c in g:
            c0_of_c[c] = g[0]
            fg_of_c[c] = fg
    rank = np.arange(N_ATOMS, dtype=np.int64)
    core = rank % N_CORES
    r = rank // N_CORES
    c_of = r // (P * CAT)
    w = r % (P * CAT)
    p_of = w % P
    j_of = w // P
    row = core * P + p_of
    colE = c_of * CAT + j_of
    kc = np.asarray(Ks, np.int64)[c_of]
    dcol = offs_c[c_of] + offs_c[c0_of_c[c_of]]
    based = row * 2 * W + dcol + j_of * kc

    row_of = np.empty(N_ATOMS, np.int64)
    colE_of = np.empty(N_ATOMS, np.int64)
    based_of = np.empty(N_ATOMS, np.int64)
    fc_of = np.empty(N_ATOMS, np.int64)
    row_of[a_order] = row
    colE_of[a_order] = colE
    based_of[a_order] = based
    fc_of[a_order] = fg_of_c[c_of]

    e_order = np.argsort(ii_sub, kind="stable")
    i_s = ii_sub[e_order]
    csr = np.zeros(N_ATOMS, np.int64)
    np.cumsum(counts[:-1], out=csr[1:])
    slot = np.arange(len(ii_sub), dtype=np.int64) - csr[i_s]
    pos_d = based_of[i_s] + slot
    pos_q = pos_d + fc_of[i_s]

    dq = np.zeros((N_CORES * P, 2 * W), np.float32)
    for g in groups:
        o = 2 * int(offs_c[g[0]])
        fg = sum(CAT * Ks[c] for c in g)
        dq[:, o:o + fg] = 1.0            # d padding (avoid ln(0))
    dq = dq.reshape(-1)
    dq[pos_d] = d_sub[e_order]
    dq[pos_q] = qj_sub[e_order]
    return {
        "dq": dq.reshape(N_CORES * P, 2 * W),
        "Ks": Ks,
        "row_of": row_of,
        "colE_of": colE_of,
    }


def _prep(qi, edge_dist, edge_index, q_ref, N, atom_mol_batch):
    qi = np.asarray(qi, np.float32)
    edge_dist = np.asarray(edge_dist, np.float32)
    ii = np.asarray(edge_index[0], np.int64)
    jj = np.asarray(edge_index[1], np.int64)
    # charge-neutrality correction (index-driven segment sum over atoms)
    q_mol = np.bincount(np.asarray(atom_mol_batch, np.int64), weights=qi,
                        minlength=N_MOL).astype(np.float32)
    corr = (q_mol - np.asarray(q_ref, np.float32)) / np.asarray(N, np.float32)
    qi_c = qi - corr[np.asarray(atom_mol_batch, np.int64)]
    qj_c = qi_c[jj]

    near = edge_dist < (CUTOFF / 2.0)
    farm = ~near
    LN = _stream_layout(ii[near], edge_dist[near], qj_c[near])
    LF = _stream_layout(ii[farm], edge_dist[farm], qj_c[farm])

    # qic grids pre-scaled by +0.5*qi_c (0.5 = double-counting factor; the
    # NR reciprocal is positive, unlike v1's negated form)
    qic = qi_c * np.float32(0.5)
    qicN = np.zeros((N_CORES * P, APP), np.float32)
    qicF = np.zeros((N_CORES * P, APP), np.float32)
    qicN[LN["row_of"], LN["colE_of"]] = qic
    qicF[LF["row_of"], LF["colE_of"]] = qic
    return {
        "dqN": LN["dq"], "dqF": LF["dq"],
        "KsN": LN["Ks"], "KsF": LF["Ks"],
        "qicN": qicN, "qicF": qicF,
        "rowN": LN["row_of"], "colN": LN["colE_of"],
        "rowF": LF["row_of"], "colF": LF["colE_of"],
    }


def _shard_maps(prep):
    in_maps = []
    for c in range(N_CORES):
        rs = slice(c * P, (c + 1) * P)
        in_maps.append({
            "dqN": prep["dqN"][rs],
            "dqF": prep["dqF"][rs],
            "qicN": prep["qicN"][rs],
            "qicF": prep["qicF"][rs],
        })
    return in_maps


def _unshard(prep, res):
    eN = np.concatenate([r["EN"].reshape(P, APP) for r in res], axis=0)
    eF = np.concatenate([r["EF"].reshape(P, APP) for r in res], axis=0)
    out = eN[prep["rowN"], prep["colN"]] + eF[prep["rowF"], prep["colF"]]
    return out.astype(np.float32)


def kernel(qi, edge_dist, edge_index, q_ref, N, atom_mol_batch):
    prep = _prep(qi, edge_dist, edge_index, q_ref, N, atom_mol_batch)
    runner = _get_runner(prep["KsN"], prep["KsF"])
    maps = _shard_maps(prep)
    # Deterministic computation: rerun until two consecutive results agree
    # bit-exactly (the axon tunnel occasionally corrupts a dispatch).
    prev = None
    for _ in range(5):
        args = runner.put_inputs(maps)
        res = runner.results(runner.run(args))
        out = _unshard(prep, res)
        if prev is not None and np.array_equal(out, prev):
            return out
        prev = out
    return out


# revision 11
# speedup vs baseline: 1.2685x; 1.0197x over previous
"""CoulombLayer Trainium2 kernel v2 (8 NeuronCores, SPMD via bass).

Strategy vs v1 (baseline):
  * Edges are split on host into NEAR (d < 5, inside the smooth-cutoff
    transition of f(2d) with cutoff=10) and FAR (d >= 5, where the PhysNet
    cutoff f is exactly 0 so chi(d) = 1/d exactly).  69% of edges are far
    and need only a reciprocal and a multiply - no sqrt / smoothstep math.
  * Each stream gets its own vertex-cut CSR layout (degree-sorted atoms,
    dealt round-robin to cores, chunked with shared slot width K_c), its own
    accumulator grid and its own output; the host adds the two grids while
    unsharding (index-driven gather it already does anyway).
  * Reciprocals run as: ACT seed y = exp(-ln x) (natural_log_exp table set)
    + ONE fused Newton step on DVE via the custom-DVE op
    RECIPROCAL_APPROX_NR: ir = (2 - x*y)*y  (1 instruction instead of 2).
  * The near smoothstep g = f(z)*dm, f = z^3*((sqrt6 z - c)^2 + 0.625), is
    ONE authored custom-DVE instruction (F_SMOOTH_MUL, 8 ALU stages), with
    z = relu(1 - d/5) from ACT and dm = d - s from GPSIMD.
  * Engine balance per core (est): DVE ~52us, ACT ~50us, GPSIMD ~49us,
    DMA ~48us (fp32 memory floor).  Everything fp32: the 2e-2 max-rel-err
    gate is dominated by atoms with catastrophic term cancellation, which
    demands ~1e-7 per-term accuracy - no fp16/bf16 and no approximations
    beyond few-ulp.
  * ACT table sets: phase 1 (near s/z/p math) uses sqrt_and_others; phase 2
    (all seeds) uses natural_log_exp_and_others -> exactly 2 table loads per
    execution.
"""

import json as _json
import numpy as np

N_CORES = 8
N_ATOMS = 500_000
N_MOL = 5_000
N_EDGES = 16_000_000
CUTOFF = 10.0
P = 125                 # SBUF partitions used (125 * 500 = 62500 atoms/core)
APP = 500               # atoms per partition
CAT = 25                # atoms per compute tile (per partition)
GCAP = 2050             # max elements per grouped elementwise op

SQ6 = 6.0 ** 0.5
C15 = 15.0 / (2.0 * SQ6)

_RUNNER_CACHE = {}


# ---------------------------------------------------------------------------
# authored custom DVE ops (registered into concourse.dve_ops at first use):
#   F_SMOOTH_MUL_ANT: g = f(z)*dm, f = ((sqrt6*z - c)^2 + 0.625)*z^3 (the
#     PhysNet quintic smoothstep in z = relu(1 - d/5) form). 8 ALU stages,
#     one DVE pass instead of TT+STT+TT plus two ACT squares.
#   RECIP_NR2_ANT: two fused Newton steps y <- y*(2 - x*y) (6 stages). From
#     the ~1.2e-3 ACT exp(-ln x) seed this converges to the fp32 rounding
#     floor (~1.4e-7) in ONE DVE pass - the single-NR version left ~1e-6
#     per-term error, which the max-rel-err gate amplifies via cancellation.
# ---------------------------------------------------------------------------
_CUSTOM_OPS = None


def _get_custom_ops():
    global _CUSTOM_OPS
    if _CUSTOM_OPS is not None:
        return _CUSTOM_OPS
    import concourse.dve_ops as dve_ops
    from concourse.dve_spec import Spec, Src0, Src1, C0, C1, C2, lower, sq
    from concourse.dve_uop import DveOpSpec

    def _register(name, spec):
        if name in dve_ops._SUB_OPCODE_FOR_NAME:
            return next(op for op in dve_ops.OPS if op.name == name)
        row = max(dve_ops._SUB_OPCODE_FOR_NAME.values()) + 1
        assert row < 0x20
        shas = {}
        for ver in ("v3", "v4"):
            uops = lower(spec, ver=ver)
            shas[ver] = DveOpSpec(name=name, opcode=row, uops=uops,
                                  rd1_en=True).sha(ver)
        op = dve_ops.DveOp(name=name, spec=spec, subdim=False, uops_sha=shas)
        dve_ops._SUB_OPCODE_FOR_NAME[name] = row
        dve_ops.OPS.append(op)
        dve_ops.CUSTOM_DVE_SPECS[name] = spec
        return op

    def _fmul_ref(in0, in1, s0, s1, imm2):
        z = in0.astype(np.float32)
        f = ((z * s0 - s1) ** 2 + imm2) * z * z * z
        return (f * in1).astype(np.float32)

    fmul = _register("F_SMOOTH_MUL_ANT", Spec(
        body=(sq(Src0 * C0 - C1) + C2) * sq(Src0) * Src0 * Src1,
        reference=_fmul_ref,
    ))

    def _nr2_ref(in0, in1, s0, s1, imm2):
        t = ((s0 - in0 * in1) * in1).astype(np.float32)
        return ((s0 - in0 * t) * t).astype(np.float32)

    _y1 = (C0 - Src0 * Src1) * Src1
    nr2 = _register("RECIP_NR2_ANT", Spec(
        body=(C0 - Src0 * _y1) * _y1,
        reference=_nr2_ref,
    ))
    _CUSTOM_OPS = (fmul, nr2)
    return _CUSTOM_OPS


# ---------------------------------------------------------------------------
# walrus compat: this build rejects >1 sync-wait per instruction.  Split
# overflow waits onto NoOps inserted immediately before, same engine/block.
# ---------------------------------------------------------------------------
def _fix_bir_json(bir_json):
    m = _json.loads(bir_json)
    for fn in m.get("functions", []):
        for blk in fn.get("blocks", []):
            out = []
            for inst in blk.get("instructions", []):
                si = inst.get("sync_info")
                waits = (si or {}).get("on_wait", [])
                if len(waits) > 1:
                    for k, w in enumerate(waits[:-1]):
                        out.append({
                            "debug": inst.get("debug", 0),
                            "engine": inst["engine"],
                            "ins": [],
                            "name": f"{inst['name']}-sw{k}",
                            "opcode": "NoOp",
                            "outs": [],
                            "sync_info": {"on_update": [], "on_wait": [w]},
                        })
                    si["on_wait"] = [waits[-1]]
                out.append(inst)
            blk["instructions"] = out
    return _json.dumps(m).encode()


_PATCHED = False


def _install_compat():
    global _PATCHED
    if _PATCHED:
        return
    _PATCHED = True
    import concourse.bass_utils as bu
    import concourse.bass2jax as b2j
    orig = bu.compile_bir_kernel

    def patched(bir_json, tmpdir, neff_name="file.neff"):
        return orig(_fix_bir_json(bir_json), tmpdir, neff_name)

    bu.compile_bir_kernel = patched
    b2j.compile_bir_kernel = patched


def _groups(Ks):
    """Greedy-pack consecutive chunks into groups of total width <= GCAP."""
    out = []
    cur = [0]
    w = CAT * Ks[0]
    for c in range(1, len(Ks)):
        f = CAT * Ks[c]
        if w + f <= GCAP:
            cur.append(c)
            w += f
        else:
            out.append(cur)
            cur = [c]
            w = f
    out.append(cur)
    return out


# ---------------------------------------------------------------------------
# device program
# ---------------------------------------------------------------------------
def _build_nc(KsN, KsF, reps=1, body_mult=1,
              p_on_act=False, dm_on_gps=True, yn_on_gps=True, yf_on_gps=True,
              seed_dve=False, dma_only=False, tiny_dma=False,
              io_bufs=3, tmp_bufs=2):
    """Two-stream device program for one core.

    KsN/KsF: per-chunk slot widths for the near / far CSR layouts."""
    import concourse.bass as bass
    import concourse.mybir as mybir
    import concourse.tile as tile

    fmul, nr2 = _get_custom_ops()

    WN = sum(CAT * k for k in KsN)
    WF = sum(CAT * k for k in KsF)
    offsN = np.cumsum([0] + [CAT * k for k in KsN])
    offsF = np.cumsum([0] + [CAT * k for k in KsF])
    nc = bass.Bass()

    # const-AP pool entries for non-Copy activation biases (only 0.0/1.0 are
    # pre-registered)
    need_consts = [-0.25, 0.5] if p_on_act else []
    for v in need_consts:
        if (mybir.dt.float32, v) not in nc.const_aps.aps:
            ct = nc.alloc_sbuf_tensor(f"const-float32-{v}", [128, 1],
                                      mybir.dt.float32)
            nc.gpsimd.memset(ct.ap(), v)
            nc.const_aps.aps[(mybir.dt.float32, v)] = ct.ap()
    if need_consts:
        nc.all_engine_barrier()

    dqN_in = nc.declare_dram_parameter("dqN", [P, 2 * WN], mybir.dt.float32,
                                       isOutput=False)
    dqF_in = nc.declare_dram_parameter("dqF", [P, 2 * WF], mybir.dt.float32,
                                       isOutput=False)
    qicN_in = nc.declare_dram_parameter("qicN", [P, APP], mybir.dt.float32,
                                        isOutput=False)
    qicF_in = nc.declare_dram_parameter("qicF", [P, APP], mybir.dt.float32,
                                        isOutput=False)
    eN_out = nc.declare_dram_parameter("EN", [P, APP], mybir.dt.float32,
                                       isOutput=True)
    eF_out = nc.declare_dram_parameter("EF", [P, APP], mybir.dt.float32,
                                       isOutput=True)

    AL = mybir.AluOpType
    AF = mybir.ActivationFunctionType

    groupsN = _groups(KsN)
    groupsF = _groups(KsF)

    with tile.TileContext(nc, num_cores=N_CORES) as tc:
        with tc.tile_pool(name="io", bufs=io_bufs) as io, \
             tc.tile_pool(name="tmp", bufs=tmp_bufs) as tp, \
             tc.tile_pool(name="keep", bufs=len(groupsN)) as kp, \
             tc.tile_pool(name="accp", bufs=2) as ap_pool, \
             tc.tile_pool(name="qicp", bufs=1) as qp_pool:
            qic2 = qp_pool.tile([P, 2 * APP], mybir.dt.float32, tag="qic2")
            nc.scalar.dma_start(qic2[:, :APP], qicN_in[:])
            nc.scalar.dma_start(qic2[:, APP:], qicF_in[:])
            def body():
                # fresh accumulator generation each iteration (bufs=2): the
                # next iteration's reduces don't wait for this iteration's
                # final scale + output DMA
                acc2 = ap_pool.tile([P, 2 * APP], mybir.dt.float32, tag="acc2")
                # ---- phase 1: near geometry (sqrt_and_others table set) ----
                ph1 = []  # per near group: (p, num, Qt, width, chunks)
                fence_srcs = []
                for grp in groupsN:
                    F = sum(CAT * KsN[c] for c in grp)
                    o = 2 * int(offsN[grp[0]])
                    Dt = io.tile([P, F], mybir.dt.float32, tag="D")
                    Qt = kp.tile([P, F], mybir.dt.float32, tag="QN")
                    if tiny_dma:
                        nc.sync.dma_start(Dt[:, :64], dqN_in[:, o:o + 64])
                        nc.gpsimd.dma_start(Qt[:, :64],
                                            dqN_in[:, o + F:o + F + 64])
                    else:
                        nc.sync.dma_start(Dt[:], dqN_in[:, o:o + F])
                        nc.gpsimd.dma_start(Qt[:], dqN_in[:, o + F:o + 2 * F])
                    if dma_only:
                        ph1.append((None, None, Qt, F, grp))
                        continue
                    D = Dt[:]
                    t = tp.tile([P, F], mybir.dt.float32, tag="t")
                    z = tp.tile([P, F], mybir.dt.float32, tag="z")
                    dm = tp.tile([P, F], mybir.dt.float32, tag="dm")
                    p = kp.tile([P, F], mybir.dt.float32, tag="p")
                    num = kp.tile([P, F], mybir.dt.float32, tag="num")

                    nc.scalar.activation(t[:], D, AF.Square)
                    if p_on_act:
                        # d*s = sqrt((d^2+0.5)^2 - 0.25)
                        nc.scalar.activation(p[:], t[:], AF.Square, bias=0.5)
                        nc.scalar.activation(p[:], p[:], AF.Sqrt, bias=-0.25)
                    # near stream has d < 5 strictly (padding d=1), so
                    # z = 1 - d/5 > 0 always - no relu needed
                    nc.vector.tensor_scalar(z[:], D, -2.0 / CUTOFF, 1.0,
                                            op0=AL.mult, op1=AL.add)
                    # s overwrites t in place (ACT in-place is safe)
                    nc.scalar.activation(t[:], t[:], AF.Sqrt, bias=1.0)
                    s = t
                    if not p_on_act:
                        nc.gpsimd.tensor_tensor(p[:], D, s[:], op=AL.mult)
                    if dm_on_gps:
                        nc.gpsimd.tensor_tensor(dm[:], D, s[:], op=AL.subtract)
                    else:
                        nc.vector.tensor_tensor(dm[:], D, s[:], op=AL.subtract)
                    # g1 = f(z) * dm in one fused DVE pass (in place over z)
                    nc.vector._custom_dve(fmul, out=z[:], in0=z[:], in1=dm[:],
                                          s0=SQ6, s1=C15, imm2=0.625)
                    nc.vector.tensor_tensor(num[:], z[:], s[:], op=AL.add)
                    ph1.append((p, num, Qt, F, grp))
                    fence_srcs.append(s)

                # ---- phase 2: seeds + Newton + scatter (natural_log_exp) ----
                # Fence: a [P,1] exact-0.0 tile whose producer chain reads one
                # column of every phase-1 Sqrt output.  Used as the bias AP of
                # every Ln, it forces the scheduler to keep ALL sqrt-set ACT
                # ops before ALL natural_log-set ops (the engine runs in
                # order, so interleaving would re-load tables ~2.7us a pop).
                fence = None
                if not dma_only:
                    for s_t in fence_srcs:
                        fence_new = tp.tile([P, 1], mybir.dt.float32,
                                            tag="fence")
                        nc.scalar.activation(
                            fence_new[:], s_t[:, 0:1],
                            AF.Identity, scale=0.0,
                            bias=fence[:] if fence is not None else 0.0)
                        fence = fence_new
                for p, num, Qt, F, grp in ph1:
                    if dma_only:
                        loc = 0
                        for c in grp:
                            K = KsN[c]
                            Fc = CAT * K
                            nc.vector.tensor_reduce(
                                acc2[:, c * CAT:(c + 1) * CAT],
                                Qt[:, loc:loc + Fc].rearrange(
                                    "p (a k) -> p a k", k=K),
                                axis=mybir.AxisListType.X,
                                op=AL.add,
                            )
                            loc += Fc
                        continue
                    y = tp.tile([P, F], mybir.dt.float32, tag="y")
                    Yt = tp.tile([P, F], mybir.dt.float32, tag="Y")
                    if seed_dve:
                        nc.vector.reciprocal_approx_fast(y[:], p[:])
                    else:
                        nc.scalar.activation(y[:], p[:], AF.Ln,
                                             bias=fence[:] if fence is not None
                                             else 0.0)
                        nc.scalar.activation(y[:], y[:], AF.Exp, scale=-1.0)
                    # ir = double-Newton 1/p, in place over y
                    nc.vector._custom_dve(nr2, out=y[:],
                                          in0=p[:], in1=y[:], s0=2.0)
                    if yn_on_gps:
                        nc.gpsimd.tensor_tensor(Yt[:], y[:], Qt[:], op=AL.mult)
                    else:
                        nc.vector.tensor_tensor(Yt[:], y[:], Qt[:], op=AL.mult)
                    nc.vector.tensor_tensor(Yt[:], num[:], Yt[:], op=AL.mult)
                    loc = 0
                    for c in grp:
                        K = KsN[c]
                        Fc = CAT * K
                        nc.vector.tensor_reduce(
                            acc2[:, c * CAT:(c + 1) * CAT],
                            Yt[:, loc:loc + Fc].rearrange(
                                "p (a k) -> p a k", k=K),
                            axis=mybir.AxisListType.X,
                            op=AL.add,
                        )
                        loc += Fc

                for grp in groupsF:
                    F = sum(CAT * KsF[c] for c in grp)
                    o = 2 * int(offsF[grp[0]])
                    Dt = io.tile([P, F], mybir.dt.float32, tag="D")
                    Qt = io.tile([P, F], mybir.dt.float32, tag="Q")
                    if tiny_dma:
                        nc.sync.dma_start(Dt[:, :64], dqF_in[:, o:o + 64])
                        nc.gpsimd.dma_start(Qt[:, :64],
                                            dqF_in[:, o + F:o + F + 64])
                    else:
                        nc.sync.dma_start(Dt[:], dqF_in[:, o:o + F])
                        nc.gpsimd.dma_start(Qt[:], dqF_in[:, o + F:o + 2 * F])
                    D = Dt[:]
                    if dma_only:
                        loc = 0
                        for c in grp:
                            K = KsF[c]
                            Fc = CAT * K
                            nc.vector.tensor_reduce(
                                acc2[:, APP + c * CAT:APP + (c + 1) * CAT],
                                Qt[:, loc:loc + Fc].rearrange(
                                    "p (a k) -> p a k", k=K),
                                axis=mybir.AxisListType.X,
                                op=AL.add,
                            )
                            loc += Fc
                        continue
                    y = tp.tile([P, F], mybir.dt.float32, tag="y")
                    Yt = tp.tile([P, F], mybir.dt.float32, tag="Y")
                    if seed_dve:
                        nc.vector.reciprocal_approx_fast(y[:], D)
                    else:
                        nc.scalar.activation(y[:], D, AF.Ln,
                                             bias=fence[:] if fence is not None
                                             else 0.0)
                        nc.scalar.activation(y[:], y[:], AF.Exp, scale=-1.0)
                    nc.vector._custom_dve(nr2, out=y[:],
                                          in0=D, in1=y[:], s0=2.0)
                    if yf_on_gps:
                        nc.gpsimd.tensor_tensor(Yt[:], y[:], Qt[:], op=AL.mult)
                    else:
                        nc.vector.tensor_tensor(Yt[:], y[:], Qt[:], op=AL.mult)
                    loc = 0
                    for c in grp:
                        K = KsF[c]
                        Fc = CAT * K
                        nc.vector.tensor_reduce(
                            acc2[:, APP + c * CAT:APP + (c + 1) * CAT],
                            Yt[:, loc:loc + Fc].rearrange(
                                "p (a k) -> p a k", k=K),
                            axis=mybir.AxisListType.X,
                            op=AL.add,
                        )
                        loc += Fc

                # E = acc * qic (qic pre-scaled by +0.5*qi_c on host)
                nc.vector.tensor_tensor(acc2[:], acc2[:], qic2[:], op=AL.mult)
                nc.scalar.dma_start(eN_out[:], acc2[:, :APP])
                nc.scalar.dma_start(eF_out[:], acc2[:, APP:])

            if reps == 1:
                body()
            else:
                with tc.For_i(0, reps):
                    for _ in range(body_mult):
                        body()
    # populate .instr bytes for InstISA subclasses (custom DVE ops); without
    # this walrus fails with "ISA wrong length"
    mybir.codegen_inst_isa_subclasses(nc)
    return nc


class _Runner:
    """Compile once; keep a reusable jitted SPMD callable."""

    def __init__(self, nc):
        import jax
        from jax.sharding import Mesh, PartitionSpec, NamedSharding
        from jax.experimental.shard_map import shard_map
        import concourse.mybir as mybir
        import concourse.bass2jax as b2j
        b2j.install_neuronx_cc_hook()
        self.jax = jax
        in_names, out_names, out_avals, zero_outs = [], [], [], []
        pname = nc.partition_id_tensor.name if nc.partition_id_tensor else None
        for alloc in nc.m.functions[0].allocations:
            if not isinstance(alloc, mybir.MemoryLocationSet):
                continue
            name = alloc.memorylocations[0].name
            if alloc.kind == "ExternalInput":
                if name != pname:
                    in_names.append(name)
            elif alloc.kind == "ExternalOutput":
                shape = tuple(alloc.tensor_shape)
                dtype = mybir.dt.np(alloc.dtype)
                out_names.append(name)
                out_avals.append(jax.core.ShapedArray(shape, dtype))
                zero_outs.append(np.zeros(shape, dtype))
        self.in_names, self.out_names = in_names, out_names
        self.out_avals, self.zero_outs = out_avals, zero_outs
        all_in = list(in_names) + list(out_names) + ([pname] if pname else [])

        def _body(*args):
            operands = list(args)
            if pname is not None:
                operands.append(b2j.partition_id_tensor())
            return tuple(b2j._bass_exec_p.bind(
                *operands,
                out_avals=tuple(out_avals),
                in_names=tuple(all_in),
                out_names=tuple(out_names),
                lowering_input_output_aliases=(),
                sim_require_finite=True,
                sim_require_nnan=True,
                nc=nc,
            ))

        devices = jax.devices()[:N_CORES]
        mesh = Mesh(np.asarray(devices), ("core",))
        n_in = len(in_names) + len(zero_outs)
        self.fn = jax.jit(
            shard_map(_body, mesh=mesh,
                      in_specs=(PartitionSpec("core"),) * n_in,
                      out_specs=(PartitionSpec("core"),) * len(out_names),
                      check_rep=False),
            keep_unused=True,
        )
        self.sharding = NamedSharding(mesh, PartitionSpec("core"))

    def put_inputs(self, in_maps, device_resident=False):
        args = []
        for name in self.in_names:
            cat = np.concatenate([np.asarray(m[name]) for m in in_maps], axis=0)
            args.append(cat)
        for z in self.zero_outs:
            args.append(np.zeros((N_CORES * z.shape[0], *z.shape[1:]), z.dtype))
        if device_resident:
            try:
                jax = self.jax
                devices = list(self.sharding.mesh.devices.reshape(-1))
                put = []
                for a in args:
                    per = a.shape[0] // N_CORES
                    shards = [
                        jax.device_put(a[c * per:(c + 1) * per], devices[c])
                        for c in range(N_CORES)
                    ]
                    put.append(jax.make_array_from_single_device_arrays(
                        a.shape, self.sharding, shards))
                jax.block_until_ready(put)
                args = put
            except Exception:
                pass
        return args

    def run(self, args):
        outs = self.fn(*args)
        self.jax.block_until_ready(outs)
        return outs

    def results(self, outs):
        res = []
        for c in range(N_CORES):
            res.append({
                name: np.asarray(outs[i]).reshape(N_CORES, *self.out_avals[i].shape)[c]
                for i, name in enumerate(self.out_names)
            })
        return res


def _get_runner(KsN, KsF, reps=1, body_mult=1, **bk):
    key = (tuple(KsN), tuple(KsF), reps, body_mult, tuple(sorted(bk.items())))
    if key not in _RUNNER_CACHE:
        _install_compat()
        _RUNNER_CACHE[key] = _Runner(
            _build_nc(tuple(KsN), tuple(KsF), reps, body_mult, **bk))
    return _RUNNER_CACHE[key]


# ---------------------------------------------------------------------------
# host-side shard construction: one degree-bucketed CSR layout per stream
# ---------------------------------------------------------------------------
def _stream_layout(ii_sub, d_sub, qj_sub):
    """Build the [N_CORES*P, 2W] interleaved (d, qj) grid + per-atom output
    position maps for one edge subset (edges targeting atom ii_sub)."""
    counts = np.bincount(ii_sub, minlength=N_ATOMS)
    a_order = np.argsort(-counts, kind="stable")
    degs = counts[a_order]
    n_chunks = APP // CAT
    cg = N_CORES * P * CAT
    Ks = tuple(int(degs[c * cg:(c + 1) * cg].max()) for c in range(n_chunks))
    W = sum(CAT * k for k in Ks)
    offs_c = np.cumsum([0] + [CAT * k for k in Ks])

    groups = _groups(Ks)
    c0_of_c = np.empty(n_chunks, np.int64)
    fg_of_c = np.empty(n_chunks, np.int64)
    for g in groups:
        fg = sum(CAT * Ks[c] for c in g)
        for c in g:
            c0_of_c[c] = g[0]
            fg_of_c[c] = fg
    rank = np.arange(N_ATOMS, dtype=np.int64)
    core = rank % N_CORES
    r = rank // N_CORES
    c_of = r // (P * CAT)
    w = r % (P * CAT)
    p_of = w % P
    j_of = w // P
    row = core * P + p_of
    colE = c_of * CAT + j_of
    kc = np.asarray(Ks, np.int64)[c_of]
    dcol = offs_c[c_of] + offs_c[c0_of_c[c_of]]
    based = row * 2 * W + dcol + j_of * kc

    row_of = np.empty(N_ATOMS, np.int64)
    colE_of = np.empty(N_ATOMS, np.int64)
    based_of = np.empty(N_ATOMS, np.int64)
    fc_of = np.empty(N_ATOMS, np.int64)
    row_of[a_order] = row
    colE_of[a_order] = colE
    based_of[a_order] = based
    fc_of[a_order] = fg_of_c[c_of]

    e_order = np.argsort(ii_sub, kind="stable")
    i_s = ii_sub[e_order]
    csr = np.zeros(N_ATOMS, np.int64)
    np.cumsum(counts[:-1], out=csr[1:])
    slot = np.arange(len(ii_sub), dtype=np.int64) - csr[i_s]
    pos_d = based_of[i_s] + slot
    pos_q = pos_d + fc_of[i_s]

    dq = np.zeros((N_CORES * P, 2 * W), np.float32)
    for g in groups:
        o = 2 * int(offs_c[g[0]])
        fg = sum(CAT * Ks[c] for c in g)
        dq[:, o:o + fg] = 1.0            # d padding (avoid ln(0))
    dq = dq.reshape(-1)
    dq[pos_d] = d_sub[e_order]
    dq[pos_q] = qj_sub[e_order]
    return {
        "dq": dq.reshape(N_CORES * P, 2 * W),
        "Ks": Ks,
        "row_of": row_of,
        "colE_of": colE_of,
    }


def _prep(qi, edge_dist, edge_index, q_ref, N, atom_mol_batch):
    qi = np.asarray(qi, np.float32)
    edge_dist = np.asarray(edge_dist, np.float32)
    ii = np.asarray(edge_index[0], np.int64)
    jj = np.asarray(edge_index[1], np.int64)
    # charge-neutrality correction (index-driven segment sum over atoms)
    q_mol = np.bincount(np.asarray(atom_mol_batch, np.int64), weights=qi,
                        minlength=N_MOL).astype(np.float32)
    corr = (q_mol - np.asarray(q_ref, np.float32)) / np.asarray(N, np.float32)
    qi_c = qi - corr[np.asarray(atom_mol_batch, np.int64)]
    qj_c = qi_c[jj]

    near = edge_dist < (CUTOFF / 2.0)
    farm = ~near
    LN = _stream_layout(ii[near], edge_dist[near], qj_c[near])
    LF = _stream_layout(ii[farm], edge_dist[farm], qj_c[farm])

    # qic grids pre-scaled by +0.5*qi_c (0.5 = double-counting factor; the
    # NR reciprocal is positive, unlike v1's negated form)
    qic = qi_c * np.float32(0.5)
    qicN = np.zeros((N_CORES * P, APP), np.float32)
    qicF = np.zeros((N_CORES * P, APP), np.float32)
    qicN[LN["row_of"], LN["colE_of"]] = qic
    qicF[LF["row_of"], LF["colE_of"]] = qic
    return {
        "dqN": LN["dq"], "dqF": LF["dq"],
        "KsN": LN["Ks"], "KsF": LF["Ks"],
        "qicN": qicN, "qicF": qicF,
        "rowN": LN["row_of"], "colN": LN["colE_of"],
        "rowF": LF["row_of"], "colF": LF["colE_of"],
    }


def _shard_maps(prep):
    in_maps = []
    for c in range(N_CORES):
        rs = slice(c * P, (c + 1) * P)
        in_maps.append({
            "dqN": prep["dqN"][rs],
            "dqF": prep["dqF"][rs],
            "qicN": prep["qicN"][rs],
            "qicF": prep["qicF"][rs],
        })
    return in_maps


def _unshard(prep, res):
    eN = np.concatenate([r["EN"].reshape(P, APP) for r in res], axis=0)
    eF = np.concatenate([r["EF"].reshape(P, APP) for r in res], axis=0)
    out = eN[prep["rowN"], prep["colN"]] + eF[prep["rowF"], prep["colF"]]
    return out.astype(np.float32)


def kernel(qi, edge_dist, edge_index, q_ref, N, atom_mol_batch):
    prep = _prep(qi, edge_dist, edge_index, q_ref, N, atom_mol_batch)
    runner = _get_runner(prep["KsN"], prep["KsF"])
    maps = _shard_maps(prep)
    # Deterministic computation: rerun until two consecutive results agree
    # bit-exactly (the axon tunnel occasionally corrupts a dispatch).
    prev = None
    for _ in range(5):
        args = runner.put_inputs(maps)
        res = runner.results(runner.run(args))
        out = _unshard(prep, res)
        if prev is not None and np.array_equal(out, prev):
            return out
        prev = out
    return out


# revision 15
# speedup vs baseline: 1.2814x; 1.0102x over previous
"""CoulombLayer Trainium2 kernel v2 (8 NeuronCores, SPMD via bass).

Strategy vs v1 (baseline):
  * Edges are split on host into NEAR (d < 5, inside the smooth-cutoff
    transition of f(2d) with cutoff=10) and FAR (d >= 5, where the PhysNet
    cutoff f is exactly 0 so chi(d) = 1/d exactly).  69% of edges are far
    and need only a reciprocal and a multiply - no sqrt / smoothstep math.
  * Each stream gets its own vertex-cut CSR layout (degree-sorted atoms,
    dealt round-robin to cores, chunked with shared slot width K_c), its own
    accumulator grid and its own output; the host adds the two grids while
    unsharding (index-driven gather it already does anyway).
  * Reciprocals run as: ACT seed y = exp(-ln x) (natural_log_exp table set)
    + ONE fused Newton step on DVE via the custom-DVE op
    RECIPROCAL_APPROX_NR: ir = (2 - x*y)*y  (1 instruction instead of 2).
  * The near smoothstep g = f(z)*dm, f = z^3*((sqrt6 z - c)^2 + 0.625), is
    ONE authored custom-DVE instruction (F_SMOOTH_MUL, 8 ALU stages), with
    z = relu(1 - d/5) from ACT and dm = d - s from GPSIMD.
  * Engine balance per core (est): DVE ~52us, ACT ~50us, GPSIMD ~49us,
    DMA ~48us (fp32 memory floor).  Everything fp32: the 2e-2 max-rel-err
    gate is dominated by atoms with catastrophic term cancellation, which
    demands ~1e-7 per-term accuracy - no fp16/bf16 and no approximations
    beyond few-ulp.
  * ACT table sets: phase 1 (near s/z/p math) uses sqrt_and_others; phase 2
    (all seeds) uses natural_log_exp_and_others -> exactly 2 table loads per
    execution.
"""

import json as _json
import numpy as np

N_CORES = 8
N_ATOMS = 500_000
N_MOL = 5_000
N_EDGES = 16_000_000
CUTOFF = 10.0
P = 125                 # SBUF partitions used (125 * 500 = 62500 atoms/core)
APP = 500               # atoms per partition
CAT = 25                # atoms per compute tile (per partition)
GCAP = 2050             # max elements per grouped elementwise op

SQ6 = 6.0 ** 0.5
C15 = 15.0 / (2.0 * SQ6)

_RUNNER_CACHE = {}


# ---------------------------------------------------------------------------
# authored custom DVE ops (registered into concourse.dve_ops at first use):
#   F_SMOOTH_MUL_ANT: g = f(z)*dm, f = ((sqrt6*z - c)^2 + 0.625)*z^3 (the
#     PhysNet quintic smoothstep in z = relu(1 - d/5) form). 8 ALU stages,
#     one DVE pass instead of TT+STT+TT plus two ACT squares.
#   RECIP_NR2_ANT: two fused Newton steps y <- y*(2 - x*y) (6 stages). From
#     the ~1.2e-3 ACT exp(-ln x) seed this converges to the fp32 rounding
#     floor (~1.4e-7) in ONE DVE pass - the single-NR version left ~1e-6
#     per-term error, which the max-rel-err gate amplifies via cancellation.
# ---------------------------------------------------------------------------
_CUSTOM_OPS = None


def _get_custom_ops():
    global _CUSTOM_OPS
    if _CUSTOM_OPS is not None:
        return _CUSTOM_OPS
    import concourse.dve_ops as dve_ops
    from concourse.dve_spec import Spec, Src0, Src1, C0, C1, C2, lower, sq
    from concourse.dve_uop import DveOpSpec

    def _register(name, spec):
        if name in dve_ops._SUB_OPCODE_FOR_NAME:
            return next(op for op in dve_ops.OPS if op.name == name)
        row = max(dve_ops._SUB_OPCODE_FOR_NAME.values()) + 1
        assert row < 0x20
        shas = {}
        for ver in ("v3", "v4"):
            uops = lower(spec, ver=ver)
            shas[ver] = DveOpSpec(name=name, opcode=row, uops=uops,
                                  rd1_en=True).sha(ver)
        op = dve_ops.DveOp(name=name, spec=spec, subdim=False, uops_sha=shas)
        dve_ops._SUB_OPCODE_FOR_NAME[name] = row
        dve_ops.OPS.append(op)
        dve_ops.CUSTOM_DVE_SPECS[name] = spec
        return op

    def _fmul_ref(in0, in1, s0, s1, imm2):
        z = in0.astype(np.float32)
        f = ((z * s0 - s1) ** 2 + imm2) * z * z * z
        return (f * in1).astype(np.float32)

    fmul = _register("F_SMOOTH_MUL_ANT", Spec(
        body=(sq(Src0 * C0 - C1) + C2) * sq(Src0) * Src0 * Src1,
        reference=_fmul_ref,
    ))

    def _nr2_ref(in0, in1, s0, s1, imm2):
        t = ((s0 - in0 * in1) * in1).astype(np.float32)
        return ((s0 - in0 * t) * t).astype(np.float32)

    _y1 = (C0 - Src0 * Src1) * Src1
    nr2 = _register("RECIP_NR2_ANT", Spec(
        body=(C0 - Src0 * _y1) * _y1,
        reference=_nr2_ref,
    ))
    _CUSTOM_OPS = (fmul, nr2)
    return _CUSTOM_OPS


# ---------------------------------------------------------------------------
# walrus compat: this build rejects >1 sync-wait per instruction.  Split
# overflow waits onto NoOps inserted immediately before, same engine/block.
# ---------------------------------------------------------------------------
def _fix_bir_json(bir_json):
    m = _json.loads(bir_json)
    for fn in m.get("functions", []):
        for blk in fn.get("blocks", []):
            out = []
            for inst in blk.get("instructions", []):
                si = inst.get("sync_info")
                waits = (si or {}).get("on_wait", [])
                if len(waits) > 1:
                    for k, w in enumerate(waits[:-1]):
                        out.append({
                            "debug": inst.get("debug", 0),
                            "engine": inst["engine"],
                            "ins": [],
                            "name": f"{inst['name']}-sw{k}",
                            "opcode": "NoOp",
                            "outs": [],
                            "sync_info": {"on_update": [], "on_wait": [w]},
                        })
                    si["on_wait"] = [waits[-1]]
                out.append(inst)
            blk["instructions"] = out
    return _json.dumps(m).encode()


_PATCHED = False


def _install_compat():
    global _PATCHED
    if _PATCHED:
        return
    _PATCHED = True
    import concourse.bass_utils as bu
    import concourse.bass2jax as b2j
    orig = bu.compile_bir_kernel

    def patched(bir_json, tmpdir, neff_name="file.neff"):
        return orig(_fix_bir_json(bir_json), tmpdir, neff_name)

    bu.compile_bir_kernel = patched
    b2j.compile_bir_kernel = patched


def _groups(Ks):
    """Greedy-pack consecutive chunks into groups of total width <= GCAP."""
    out = []
    cur = [0]
    w = CAT * Ks[0]
    for c in range(1, len(Ks)):
        f = CAT * Ks[c]
        if w + f <= GCAP:
            cur.append(c)
            w += f
        else:
            out.append(cur)
            cur = [c]
            w = f
    out.append(cur)
    return out


# ---------------------------------------------------------------------------
# device program
# ---------------------------------------------------------------------------
def _build_nc(KsN, KsF, reps=1, body_mult=1,
              p_on_act=False, dm_on_gps=False, yn_on_gps=True, yf_on_gps=True,
              seed_dve=False, dma_only=False, tiny_dma=False,
              io_bufs=3, tmp_bufs=2):
    """Two-stream device program for one core.

    KsN/KsF: per-chunk slot widths for the near / far CSR layouts."""
    import concourse.bass as bass
    import concourse.mybir as mybir
    import concourse.tile as tile

    fmul, nr2 = _get_custom_ops()

    WN = sum(CAT * k for k in KsN)
    WF = sum(CAT * k for k in KsF)
    offsN = np.cumsum([0] + [CAT * k for k in KsN])
    offsF = np.cumsum([0] + [CAT * k for k in KsF])
    nc = bass.Bass()

    # const-AP pool entries for non-Copy activation biases (only 0.0/1.0 are
    # pre-registered)
    need_consts = [-0.25, 0.5] if p_on_act else []
    for v in need_consts:
        if (mybir.dt.float32, v) not in nc.const_aps.aps:
            ct = nc.alloc_sbuf_tensor(f"const-float32-{v}", [128, 1],
                                      mybir.dt.float32)
            nc.gpsimd.memset(ct.ap(), v)
            nc.const_aps.aps[(mybir.dt.float32, v)] = ct.ap()
    if need_consts:
        nc.all_engine_barrier()

    dqN_in = nc.declare_dram_parameter("dqN", [P, 2 * WN], mybir.dt.float32,
                                       isOutput=False)
    dqF_in = nc.declare_dram_parameter("dqF", [P, 2 * WF], mybir.dt.float32,
                                       isOutput=False)
    qicN_in = nc.declare_dram_parameter("qicN", [P, APP], mybir.dt.float32,
                                        isOutput=False)
    qicF_in = nc.declare_dram_parameter("qicF", [P, APP], mybir.dt.float32,
                                        isOutput=False)
    e_out = nc.declare_dram_parameter("E2", [P, 2 * APP], mybir.dt.float32,
                                      isOutput=True)

    AL = mybir.AluOpType
    AF = mybir.ActivationFunctionType

    groupsN = _groups(KsN)
    groupsF = _groups(KsF)

    with tile.TileContext(nc, num_cores=N_CORES) as tc:
        with tc.tile_pool(name="io", bufs=io_bufs) as io, \
             tc.tile_pool(name="tmp", bufs=tmp_bufs) as tp, \
             tc.tile_pool(name="keep", bufs=len(groupsN)) as kp, \
             tc.tile_pool(name="accp", bufs=2) as ap_pool, \
             tc.tile_pool(name="qicp", bufs=1) as qp_pool:
            qic2 = qp_pool.tile([P, 2 * APP], mybir.dt.float32, tag="qic2")
            nc.scalar.dma_start(qic2[:, :APP], qicN_in[:])
            nc.scalar.dma_start(qic2[:, APP:], qicF_in[:])
            def body():
                # fresh accumulator generation each iteration (bufs=2): the
                # next iteration's reduces don't wait for this iteration's
                # final scale + output DMA
                acc2 = ap_pool.tile([P, 2 * APP], mybir.dt.float32, tag="acc2")
                # ---- phase 1: near geometry (sqrt_and_others table set) ----
                ph1 = []  # per near group: (p, num, Qt, width, chunks)
                fence_srcs = []
                for grp in groupsN:
                    F = sum(CAT * KsN[c] for c in grp)
                    o = 2 * int(offsN[grp[0]])
                    Dt = io.tile([P, F], mybir.dt.float32, tag="D")
                    Qt = kp.tile([P, F], mybir.dt.float32, tag="QN")
                    if tiny_dma:
                        nc.sync.dma_start(Dt[:, :64], dqN_in[:, o:o + 64])
                        nc.gpsimd.dma_start(Qt[:, :64],
                                            dqN_in[:, o + F:o + F + 64])
                    else:
                        nc.sync.dma_start(Dt[:], dqN_in[:, o:o + F])
                        nc.gpsimd.dma_start(Qt[:], dqN_in[:, o + F:o + 2 * F])
                    if dma_only:
                        ph1.append((None, None, Qt, F, grp))
                        continue
                    D = Dt[:]
                    t = tp.tile([P, F], mybir.dt.float32, tag="t")
                    z = tp.tile([P, F], mybir.dt.float32, tag="z")
                    dm = tp.tile([P, F], mybir.dt.float32, tag="dm")
                    p = kp.tile([P, F], mybir.dt.float32, tag="p")
                    num = kp.tile([P, F], mybir.dt.float32, tag="num")

                    nc.scalar.activation(t[:], D, AF.Square)
                    if p_on_act:
                        # d*s = sqrt((d^2+0.5)^2 - 0.25)
                        nc.scalar.activation(p[:], t[:], AF.Square, bias=0.5)
                        nc.scalar.activation(p[:], p[:], AF.Sqrt, bias=-0.25)
                    # near stream has d < 5 strictly (padding d=1), so
                    # z = 1 - d/5 > 0 always - no relu needed
                    nc.vector.tensor_scalar(z[:], D, -2.0 / CUTOFF, 1.0,
                                            op0=AL.mult, op1=AL.add)
                    # s overwrites t in place (ACT in-place is safe)
                    nc.scalar.activation(t[:], t[:], AF.Sqrt, bias=1.0)
                    s = t
                    if not p_on_act:
                        nc.gpsimd.tensor_tensor(p[:], D, s[:], op=AL.mult)
                    if dm_on_gps:
                        nc.gpsimd.tensor_tensor(dm[:], D, s[:], op=AL.subtract)
                    else:
                        nc.vector.tensor_tensor(dm[:], D, s[:], op=AL.subtract)
                    # g1 = f(z) * dm in one fused DVE pass (in place over z)
                    nc.vector._custom_dve(fmul, out=z[:], in0=z[:], in1=dm[:],
                                          s0=SQ6, s1=C15, imm2=0.625)
                    nc.vector.tensor_tensor(num[:], z[:], s[:], op=AL.add)
                    ph1.append((p, num, Qt, F, grp))
                    fence_srcs.append(s)

                # ---- phase 2: seeds + Newton + scatter (natural_log_exp) ----
                # Fence: a [P,1] exact-0.0 tile whose producer chain reads one
                # column of every phase-1 Sqrt output.  Used as the bias AP of
                # every Ln, it forces the scheduler to keep ALL sqrt-set ACT
                # ops before ALL natural_log-set ops (the engine runs in
                # order, so interleaving would re-load tables ~2.7us a pop).
                fence = None
                if not dma_only:
                    for s_t in fence_srcs:
                        fence_new = tp.tile([P, 1], mybir.dt.float32,
                                            tag="fence")
                        nc.scalar.activation(
                            fence_new[:], s_t[:, 0:1],
                            AF.Identity, scale=0.0,
                            bias=fence[:] if fence is not None else 0.0)
                        fence = fence_new
                for p, num, Qt, F, grp in ph1:
                    if dma_only:
                        loc = 0
                        for c in grp:
                            K = KsN[c]
                            Fc = CAT * K
                            nc.vector.tensor_reduce(
                                acc2[:, c * CAT:(c + 1) * CAT],
                                Qt[:, loc:loc + Fc].rearrange(
                                    "p (a k) -> p a k", k=K),
                                axis=mybir.AxisListType.X,
                                op=AL.add,
                            )
                            loc += Fc
                        continue
                    y = tp.tile([P, F], mybir.dt.float32, tag="y")
                    Yt = tp.tile([P, F], mybir.dt.float32, tag="Y")
                    if seed_dve:
                        nc.vector.reciprocal_approx_fast(y[:], p[:])
                    else:
                        nc.scalar.activation(y[:], p[:], AF.Ln,
                                             bias=fence[:] if fence is not None
                                             else 0.0)
                        nc.scalar.activation(y[:], y[:], AF.Exp, scale=-1.0)
                    # ir = double-Newton 1/p, in place over y
                    nc.vector._custom_dve(nr2, out=y[:],
                                          in0=p[:], in1=y[:], s0=2.0)
                    if yn_on_gps:
                        nc.gpsimd.tensor_tensor(Yt[:], y[:], Qt[:], op=AL.mult)
                    else:
                        nc.vector.tensor_tensor(Yt[:], y[:], Qt[:], op=AL.mult)
                    nc.vector.tensor_tensor(Yt[:], num[:], Yt[:], op=AL.mult)
                    loc = 0
                    for c in grp:
                        K = KsN[c]
                        Fc = CAT * K
                        nc.vector.tensor_reduce(
                            acc2[:, c * CAT:(c + 1) * CAT],
                            Yt[:, loc:loc + Fc].rearrange(
                                "p (a k) -> p a k", k=K),
                            axis=mybir.AxisListType.X,
                            op=AL.add,
                        )
                        loc += Fc

                for grp in groupsF:
                    F = sum(CAT * KsF[c] for c in grp)
                    o = 2 * int(offsF[grp[0]])
                    Dt = io.tile([P, F], mybir.dt.float32, tag="D")
                    Qt = io.tile([P, F], mybir.dt.float32, tag="Q")
                    if tiny_dma:
                        nc.sync.dma_start(Dt[:, :64], dqF_in[:, o:o + 64])
                        nc.gpsimd.dma_start(Qt[:, :64],
                                            dqF_in[:, o + F:o + F + 64])
                    else:
                        nc.sync.dma_start(Dt[:], dqF_in[:, o:o + F])
                        nc.gpsimd.dma_start(Qt[:], dqF_in[:, o + F:o + 2 * F])
                    D = Dt[:]
                    if dma_only:
                        loc = 0
                        for c in grp:
                            K = KsF[c]
                            Fc = CAT * K
                            nc.vector.tensor_reduce(
                                acc2[:, APP + c * CAT:APP + (c + 1) * CAT],
                                Qt[:, loc:loc + Fc].rearrange(
                                    "p (a k) -> p a k", k=K),
                                axis=mybir.AxisListType.X,
                                op=AL.add,
                            )
                            loc += Fc
                        continue
                    y = tp.tile([P, F], mybir.dt.float32, tag="y")
                    Yt = tp.tile([P, F], mybir.dt.float32, tag="Y")
                    if seed_dve:
                        nc.vector.reciprocal_approx_fast(y[:], D)
                    else:
                        nc.scalar.activation(y[:], D, AF.Ln,
                                             bias=fence[:] if fence is not None
                                             else 0.0)
                        nc.scalar.activation(y[:], y[:], AF.Exp, scale=-1.0)
                    nc.vector._custom_dve(nr2, out=y[:],
                                          in0=D, in1=y[:], s0=2.0)
                    if yf_on_gps:
                        nc.gpsimd.tensor_tensor(Yt[:], y[:], Qt[:], op=AL.mult)
                    else:
                        nc.vector.tensor_tensor(Yt[:], y[:], Qt[:], op=AL.mult)
                    loc = 0
                    for c in grp:
                        K = KsF[c]
                        Fc = CAT * K
                        nc.vector.tensor_reduce(
                            acc2[:, APP + c * CAT:APP + (c + 1) * CAT],
                            Yt[:, loc:loc + Fc].rearrange(
                                "p (a k) -> p a k", k=K),
                            axis=mybir.AxisListType.X,
                            op=AL.add,
                        )
                        loc += Fc

                # E = acc * qic (qic pre-scaled by +0.5*qi_c on host)
                nc.vector.tensor_tensor(acc2[:], acc2[:], qic2[:], op=AL.mult)
                nc.scalar.dma_start(e_out[:], acc2[:])

            if reps == 1:
                body()
            else:
                with tc.For_i(0, reps):
                    for _ in range(body_mult):
                        body()
    # populate .instr bytes for InstISA subclasses (custom DVE ops); without
    # this walrus fails with "ISA wrong length"
    mybir.codegen_inst_isa_subclasses(nc)
    return nc


class _Runner:
    """Compile once; keep a reusable jitted SPMD callable."""

    def __init__(self, nc):
        import jax
        from jax.sharding import Mesh, PartitionSpec, NamedSharding
        from jax.experimental.shard_map import shard_map
        import concourse.mybir as mybir
        import concourse.bass2jax as b2j
        b2j.install_neuronx_cc_hook()
        self.jax = jax
        in_names, out_names, out_avals, zero_outs = [], [], [], []
        pname = nc.partition_id_tensor.name if nc.partition_id_tensor else None
        for alloc in nc.m.functions[0].allocations:
            if not isinstance(alloc, mybir.MemoryLocationSet):
                continue
            name = alloc.memorylocations[0].name
            if alloc.kind == "ExternalInput":
                if name != pname:
                    in_names.append(name)
            elif alloc.kind == "ExternalOutput":
                shape = tuple(alloc.tensor_shape)
                dtype = mybir.dt.np(alloc.dtype)
                out_names.append(name)
                out_avals.append(jax.core.ShapedArray(shape, dtype))
                zero_outs.append(np.zeros(shape, dtype))
        self.in_names, self.out_names = in_names, out_names
        self.out_avals, self.zero_outs = out_avals, zero_outs
        all_in = list(in_names) + list(out_names) + ([pname] if pname else [])

        def _body(*args):
            operands = list(args)
            if pname is not None:
                operands.append(b2j.partition_id_tensor())
            return tuple(b2j._bass_exec_p.bind(
                *operands,
                out_avals=tuple(out_avals),
                in_names=tuple(all_in),
                out_names=tuple(out_names),
                lowering_input_output_aliases=(),
                sim_require_finite=True,
                sim_require_nnan=True,
                nc=nc,
            ))

        devices = jax.devices()[:N_CORES]
        mesh = Mesh(np.asarray(devices), ("core",))
        n_in = len(in_names) + len(zero_outs)
        self.fn = jax.jit(
            shard_map(_body, mesh=mesh,
                      in_specs=(PartitionSpec("core"),) * n_in,
                      out_specs=(PartitionSpec("core"),) * len(out_names),
                      check_rep=False),
            keep_unused=True,
        )
        self.sharding = NamedSharding(mesh, PartitionSpec("core"))

    def put_inputs(self, in_maps, device_resident=False):
        args = []
        for name in self.in_names:
            cat = np.concatenate([np.asarray(m[name]) for m in in_maps], axis=0)
            args.append(cat)
        for z in self.zero_outs:
            args.append(np.zeros((N_CORES * z.shape[0], *z.shape[1:]), z.dtype))
        if device_resident:
            try:
                jax = self.jax
                devices = list(self.sharding.mesh.devices.reshape(-1))
                put = []
                for a in args:
                    per = a.shape[0] // N_CORES
                    shards = [
                        jax.device_put(a[c * per:(c + 1) * per], devices[c])
                        for c in range(N_CORES)
                    ]
                    put.append(jax.make_array_from_single_device_arrays(
                        a.shape, self.sharding, shards))
                jax.block_until_ready(put)
                args = put
            except Exception:
                pass
        return args

    def run(self, args):
        outs = self.fn(*args)
        self.jax.block_until_ready(outs)
        return outs

    def results(self, outs):
        res = []
        for c in range(N_CORES):
            res.append({
                name: np.asarray(outs[i]).reshape(N_CORES, *self.out_avals[i].shape)[c]
                for i, name in enumerate(self.out_names)
            })
        return res


def _get_runner(KsN, KsF, reps=1, body_mult=1, **bk):
    key = (tuple(KsN), tuple(KsF), reps, body_mult, tuple(sorted(bk.items())))
    if key not in _RUNNER_CACHE:
        _install_compat()
        _RUNNER_CACHE[key] = _Runner(
            _build_nc(tuple(KsN), tuple(KsF), reps, body_mult, **bk))
    return _RUNNER_CACHE[key]


# ---------------------------------------------------------------------------
# host-side shard construction: one degree-bucketed CSR layout per stream
# ---------------------------------------------------------------------------
def _stream_layout(ii_sub, d_sub, qj_sub):
    """Build the [N_CORES*P, 2W] interleaved (d, qj) grid + per-atom output
    position maps for one edge subset (edges targeting atom ii_sub)."""
    counts = np.bincount(ii_sub, minlength=N_ATOMS)
    a_order = np.argsort(-counts, kind="stable")
    degs = counts[a_order]
    n_chunks = APP // CAT
    cg = N_CORES * P * CAT
    Ks = tuple(int(degs[c * cg:(c + 1) * cg].max()) for c in range(n_chunks))
    W = sum(CAT * k for k in Ks)
    offs_c = np.cumsum([0] + [CAT * k for k in Ks])

    groups = _groups(Ks)
    c0_of_c = np.empty(n_chunks, np.int64)
    fg_of_c = np.empty(n_chunks, np.int64)
    for g in groups:
        fg = sum(CAT * Ks[c] for c in g)
        for c in g:
            c0_of_c[c] = g[0]
            fg_of_c[c] = fg
    rank = np.arange(N_ATOMS, dtype=np.int64)
    core = rank % N_CORES
    r = rank // N_CORES
    c_of = r // (P * CAT)
    w = r % (P * CAT)
    p_of = w % P
    j_of = w // P
    row = core * P + p_of
    colE = c_of * CAT + j_of
    kc = np.asarray(Ks, np.int64)[c_of]
    dcol = offs_c[c_of] + offs_c[c0_of_c[c_of]]
    based = row * 2 * W + dcol + j_of * kc

    row_of = np.empty(N_ATOMS, np.int64)
    colE_of = np.empty(N_ATOMS, np.int64)
    based_of = np.empty(N_ATOMS, np.int64)
    fc_of = np.empty(N_ATOMS, np.int64)
    row_of[a_order] = row
    colE_of[a_order] = colE
    based_of[a_order] = based
    fc_of[a_order] = fg_of_c[c_of]

    e_order = np.argsort(ii_sub, kind="stable")
    i_s = ii_sub[e_order]
    csr = np.zeros(N_ATOMS, np.int64)
    np.cumsum(counts[:-1], out=csr[1:])
    slot = np.arange(len(ii_sub), dtype=np.int64) - csr[i_s]
    pos_d = based_of[i_s] + slot
    pos_q = pos_d + fc_of[i_s]

    dq = np.zeros((N_CORES * P, 2 * W), np.float32)
    for g in groups:
        o = 2 * int(offs_c[g[0]])
        fg = sum(CAT * Ks[c] for c in g)
        dq[:, o:o + fg] = 1.0            # d padding (avoid ln(0))
    dq = dq.reshape(-1)
    dq[pos_d] = d_sub[e_order]
    dq[pos_q] = qj_sub[e_order]
    return {
        "dq": dq.reshape(N_CORES * P, 2 * W),
        "Ks": Ks,
        "row_of": row_of,
        "colE_of": colE_of,
    }


def _prep(qi, edge_dist, edge_index, q_ref, N, atom_mol_batch):
    qi = np.asarray(qi, np.float32)
    edge_dist = np.asarray(edge_dist, np.float32)
    ii = np.asarray(edge_index[0], np.int64)
    jj = np.asarray(edge_index[1], np.int64)
    # charge-neutrality correction (index-driven segment sum over atoms)
    q_mol = np.bincount(np.asarray(atom_mol_batch, np.int64), weights=qi,
                        minlength=N_MOL).astype(np.float32)
    corr = (q_mol - np.asarray(q_ref, np.float32)) / np.asarray(N, np.float32)
    qi_c = qi - corr[np.asarray(atom_mol_batch, np.int64)]
    qj_c = qi_c[jj]

    near = edge_dist < (CUTOFF / 2.0)
    farm = ~near
    LN = _stream_layout(ii[near], edge_dist[near], qj_c[near])
    LF = _stream_layout(ii[farm], edge_dist[farm], qj_c[farm])

    # qic grids pre-scaled by +0.5*qi_c (0.5 = double-counting factor; the
    # NR reciprocal is positive, unlike v1's negated form)
    qic = qi_c * np.float32(0.5)
    qicN = np.zeros((N_CORES * P, APP), np.float32)
    qicF = np.zeros((N_CORES * P, APP), np.float32)
    qicN[LN["row_of"], LN["colE_of"]] = qic
    qicF[LF["row_of"], LF["colE_of"]] = qic
    return {
        "dqN": LN["dq"], "dqF": LF["dq"],
        "KsN": LN["Ks"], "KsF": LF["Ks"],
        "qicN": qicN, "qicF": qicF,
        "rowN": LN["row_of"], "colN": LN["colE_of"],
        "rowF": LF["row_of"], "colF": LF["colE_of"],
    }


def _shard_maps(prep):
    in_maps = []
    for c in range(N_CORES):
        rs = slice(c * P, (c + 1) * P)
        in_maps.append({
            "dqN": prep["dqN"][rs],
            "dqF": prep["dqF"][rs],
            "qicN": prep["qicN"][rs],
            "qicF": prep["qicF"][rs],
        })
    return in_maps


def _unshard(prep, res):
    e2 = np.concatenate([r["E2"].reshape(P, 2 * APP) for r in res], axis=0)
    out = (e2[prep["rowN"], prep["colN"]]
           + e2[prep["rowF"], APP + prep["colF"]])
    return out.astype(np.float32)


def kernel(qi, edge_dist, edge_index, q_ref, N, atom_mol_batch):
    prep = _prep(qi, edge_dist, edge_index, q_ref, N, atom_mol_batch)
    runner = _get_runner(prep["KsN"], prep["KsF"])
    maps = _shard_maps(prep)
    # Deterministic computation: rerun until two consecutive results agree
    # bit-exactly (the axon tunnel occasionally corrupts a dispatch).
    prev = None
    for _ in range(5):
        args = runner.put_inputs(maps)
        res = runner.results(runner.run(args))
        out = _unshard(prep, res)
        if prev is not None and np.array_equal(out, prev):
            return out
        prev = out
    return out
